# revision 1
# baseline (speedup 1.0000x reference)
"""DifferentialAttention (B=2, S=2048, D=2048, H=16, KVH=8) on 8 TRN2 NeuronCores.

Sharding: 8 cores = 2 (batch) x 4 (tensor-parallel head groups).
Core c = 4*b + r handles batch b and real heads 4r..4r+3:
  - column-parallel q/k/v projections (q heads 8r..8r+7, k heads 4r..4r+3,
    v heads 2r..2r+1), full causal differential attention for those heads,
  - row-parallel partial o_proj; host sums the 4 partials per batch.

Device math (per core), all fp32 data with float32r matmuls:
  - host passes x^T and W^T slices so every matmul contracts on partitions
  - scores computed transposed  S^T[k,q] = k . q  so exp -> AV needs no
    on-chip transposes
  - softmax without max subtraction (scores ~ N(0,1)); row sums and their
    partition broadcast fused into one all-ones matmul
  - RMS-norm folded:  out = u * rsqrt(mean_d(u^2) + eps*r1^2)  with
    u = O1 - (lam*r1/r2)*O2,  O = E@v unnormalized,  r = rowsum(E);
    subln weight and (1 - lambda_init) folded into Wo on the host;
    rsqrt computed as exp(-0.5*ln(x)) to stay on one ACT func table
  - causal masking: matmuls sliced to valid columns; 0/1 triangular mask
    multiplied into the single mixed 128x128 block per diagonal tile
  - DMA decongestion: x^T/weights packed into multi-block [128,2048]
    transfers, k/v weights resident in SBUF, RoPE rotate-half via DVE
    partition-shifted copies
"""

import math
import numpy as np

B, S, D = 2, 2048, 2048
H, KVH = 16, 8
Dh = 64
TP = 4
NCORES = 8
LAYER_IDX = 2
LAMBDA_INIT = 0.8 - 0.6 * math.exp(-0.3 * LAYER_IDX)
EPS = 1e-5
ROPE_THETA = 10000.0

_CACHE = {}


def _build_nc():
    import concourse.bass as bass  # noqa: F401
    import concourse.tile as tile
    from concourse import bacc, mybir

    F32 = mybir.dt.float32
    F32R = mybir.dt.float32r
    Act = mybir.ActivationFunctionType
    Alu = mybir.AluOpType

    nc = bacc.Bacc("TRN2", target_bir_lowering=False, debug=False)

    xT = nc.dram_tensor("xT", [D, S], F32R, kind="ExternalInput")
    wqT = nc.dram_tensor("wqT", [D, 512], F32R, kind="ExternalInput")
    wkT = nc.dram_tensor("wkT", [D, 256], F32R, kind="ExternalInput")
    wvT = nc.dram_tensor("wvT", [D, 256], F32R, kind="ExternalInput")
    woT = nc.dram_tensor("woT", [512, D], F32R, kind="ExternalInput")
    cosT_d = nc.dram_tensor("cosT", [128, S], F32, kind="ExternalInput")
    ssinT_d = nc.dram_tensor("ssinT", [128, S], F32, kind="ExternalInput")
    tri_d = nc.dram_tensor("tri", [128, 128], F32R, kind="ExternalInput")
    ones_d = nc.dram_tensor("ones", [128, 128], F32R, kind="ExternalInput")
    lam_d = nc.dram_tensor("lam", [128, 1], F32, kind="ExternalInput")
    out_d = nc.dram_tensor("out", [S, D], F32, kind="ExternalOutput")

    KD = D // 128  # 16 contraction tiles

    with tile.TileContext(nc) as tc:
        with tc.tile_pool(name="const", bufs=1) as constp, \
             tc.tile_pool(name="persist", bufs=1) as persist:

            cosT = constp.tile([128, S], F32, tag="cos")
            ssinT = constp.tile([128, S], F32, tag="ssin")
            tri = constp.tile([128, 128], F32R, tag="tri")
            ones = constp.tile([128, 128], F32R, tag="ones")
            lam = constp.tile([128, 1], F32, tag="lam")

            qT_sb = [persist.tile([128, S], F32R, tag=f"qT{m}", name=f"qT{m}")
                     for m in range(4)]
            kTd = [persist.tile([128, S], F32R, tag=f"kTd{h}", name=f"kTd{h}")
                   for h in range(4)]
            v_sb = [persist.tile([128, 256], F32R, tag=f"v{ms}", name=f"v{ms}")
                    for ms in range(16)]

            # ---------------- Phase A: projections + RoPE ----------------
            with tc.tile_pool(name="xtp", bufs=6) as xtp, \
                 tc.tile_pool(name="wstream", bufs=2) as wsp, \
                 tc.tile_pool(name="ropet", bufs=2) as rp, \
                 tc.tile_pool(name="psA", bufs=8, space="PSUM") as psA:

                def rope_core(ps, gc0):
                    """RoPE on a [128, 512] psum tile; returns (ra, rt) to add."""
                    gsl = slice(gc0, gc0 + 512)
                    qraw = rp.tile([128, 512], F32, tag="qraw", name="qraw")
                    nc.scalar.copy(qraw[:], ps[:])
                    qsw = rp.tile([128, 512], F32, tag="qsw", name="qsw")
                    for blk in range(4):
                        sb_ = (blk ^ 1) * 32
                        nc.vector.tensor_copy(
                            qsw[blk * 32:blk * 32 + 32, :], qraw[sb_:sb_ + 32, :])
                    nc.vector.tensor_mul(qraw[:], qraw[:], cosT[:, gsl])
                    nc.vector.tensor_mul(qsw[:], qsw[:], ssinT[:, gsl])
                    return qraw, qsw

                def rope_epilogue(ps, dst, gc0):
                    ra, rt = rope_core(ps, gc0)
                    nc.vector.tensor_add(dst[:, gc0:gc0 + 512], ra[:], rt[:])

                def rope_epilogue_kdup(ps, m, gc0):
                    """RoPE then duplicate each 64-row head half into kTd[2m+e]."""
                    gsl = slice(gc0, gc0 + 512)
                    ra, rt = rope_core(ps, gc0)
                    ktmp = rp.tile([128, 512], F32R, tag="ktmp", name="ktmp")
                    nc.vector.tensor_add(ktmp[:], ra[:], rt[:])
                    for e in range(2):
                        src = ktmp[e * 64:e * 64 + 64, :]
                        nc.sync.dma_start(out=kTd[2 * m + e][0:64, gsl], in_=src)
                        nc.sync.dma_start(out=kTd[2 * m + e][64:128, gsl], in_=src)

                # resident k/v weights + tables: loaded after quarter-0 q DMAs
                wk_r = []
                wv_r = []

                def load_tables_and_kv():
                    nc.sync.dma_start(out=cosT[:], in_=cosT_d[:])
                    nc.sync.dma_start(out=ssinT[:], in_=ssinT_d[:])
                    for kp in range(KD // 8):
                        t = wsp.tile([128, 2048], F32R, tag=f"wkr{kp}",
                                     name=f"wkr{kp}", bufs=1)
                        nc.sync.dma_start(
                            out=t[:].rearrange("p (eight n) -> p eight n", eight=8),
                            in_=wkT[kp * 1024:kp * 1024 + 1024, :]
                                .rearrange("(eight p) n -> p eight n", eight=8),
                        )
                        wk_r.append(t)
                        t = wsp.tile([128, 2048], F32R, tag=f"wvr{kp}",
                                     name=f"wvr{kp}", bufs=1)
                        nc.sync.dma_start(
                            out=t[:].rearrange("p (eight n) -> p eight n", eight=8),
                            in_=wvT[kp * 1024:kp * 1024 + 1024, :]
                                .rearrange("(eight p) n -> p eight n", eight=8),
                        )
                        wv_r.append(t)
                    nc.sync.dma_start(out=tri[:], in_=tri_d[:])
                    nc.sync.dma_start(out=ones[:], in_=ones_d[:])
                    nc.sync.dma_start(out=lam[:], in_=lam_d[:])

                def wk_lhsT(kd, m):
                    return wk_r[kd // 8][:, (kd % 8) * 256 + m * 128:
                                         (kd % 8) * 256 + m * 128 + 128]

                def wv_rhs(kd):
                    return wv_r[kd // 8][:, (kd % 8) * 256:(kd % 8) * 256 + 256]

                for sh in range(4):
                    c0 = 512 * sh
                    # x^T tiles: two 128-row blocks packed along free dim
                    xt4 = []
                    for kp in range(KD // 4):
                        t = xtp.tile([128, 2048], F32R, tag="xt", name=f"xt{kp}")
                        nc.sync.dma_start(
                            out=t[:].rearrange("p (four n) -> p four n", four=4),
                            in_=xT[kp * 512:kp * 512 + 512, c0:c0 + 512]
                                .rearrange("(four p) n -> p four n", four=4),
                        )
                        xt4.append(t)

                    def xt_rhs(kd):
                        return xt4[kd // 4][:, (kd % 4) * 512:(kd % 4) * 512 + 512]

                    # q projection: qT[j, s] for j in [0,512)
                    qps = {}
                    for kp in range(KD // 4):
                        wqb = wsp.tile([128, 2048], F32R, tag="wq", name="wqb")
                        nc.sync.dma_start(
                            out=wqb[:].rearrange("p (four n) -> p four n", four=4),
                            in_=wqT[kp * 512:kp * 512 + 512, :]
                                .rearrange("(four p) n -> p four n", four=4),
                        )
                        for t in range(4):
                            kd = kp * 4 + t
                            for m in range(4):
                                if kd == 0:
                                    qps[m] = psA.tile([128, 512], F32, tag="pa",
                                                      name=f"qps{m}")
                                nc.tensor.matmul(
                                    qps[m][:],
                                    wqb[:, t * 512 + m * 128:t * 512 + m * 128 + 128],
                                    xt_rhs(kd),
                                    start=(kd == 0), stop=(kd == KD - 1),
                                )
                    if sh == 0:
                        load_tables_and_kv()
                    for m in range(4):
                        rope_epilogue(qps[m], qT_sb[m], c0)

                    # k projection (4 blocks packed per DMA)
                    kps = {}
                    for kd in range(KD):
                        for m in range(2):
                            if kd == 0:
                                kps[m] = psA.tile([128, 512], F32, tag="pa",
                                                  name=f"kps{m}")
                            nc.tensor.matmul(
                                kps[m][:],
                                wk_lhsT(kd, m),
                                xt_rhs(kd),
                                start=(kd == 0), stop=(kd == KD - 1),
                            )
                    for m in range(2):
                        rope_epilogue_kdup(kps[m], m, c0)

                    # v projection: v[s, j] (s on partitions)
                    vps = {}
                    for kd in range(KD):
                        for ms in range(4):
                            if kd == 0:
                                vps[ms] = psA.tile([128, 512], F32, tag="pa",
                                                   name=f"vps{ms}")
                            nc.tensor.matmul(
                                vps[ms][:, 0:256],
                                xt_rhs(kd)[:, ms * 128:ms * 128 + 128],
                                wv_rhs(kd),
                                start=(kd == 0), stop=(kd == KD - 1),
                            )
                    for ms in range(4):
                        nc.scalar.copy(v_sb[sh * 4 + ms][:], vps[ms][:, 0:256])

            # ---------------- Phases B+C ----------------
            with tc.tile_pool(name="otfp", bufs=1) as otfp:
              otf = [otfp.tile([128, S], F32R, tag=f"otf{p}", name=f"otf{p}")
                     for p in range(4)]
              # ---------------- Phase B: attention ----------------
              with tc.tile_pool(name="etp", bufs=8) as etp, \
                 tc.tile_pool(name="ebp", bufs=8) as ebp, \
                 tc.tile_pool(name="psAcc", bufs=4, space="PSUM") as psAcc, \
                 tc.tile_pool(name="psS", bufs=4, space="PSUM") as psS:

                for p, qi in [(pp, qq) for pp in range(4) for qq in range(4)]:
                    if True:
                        vh = p // 2
                        q0 = 512 * qi
                        nki = 4 * qi + 4
                        OT1 = psAcc.tile([128, 512], F32, tag="acc", name="OT1")
                        OT2 = psAcc.tile([128, 512], F32, tag="acc", name="OT2")
                        R1 = psAcc.tile([128, 512], F32, tag="acc", name="R1")
                        R2 = psAcc.tile([128, 512], F32, tag="acc", name="R2")
                        for ki in list(range(4 * qi, nki)) + list(range(4 * qi)):
                            j = ki - 4 * qi
                            vc = 128 * j if j > 0 else 0
                            ksl = slice(ki * 128, ki * 128 + 128)
                            st = (ki == 4 * qi)
                            sp_ = (ki == 4 * qi - 1) if qi > 0 else (ki == nki - 1)
                            S1 = psS.tile([128, 512], F32, tag="s", name="S1")
                            S2 = psS.tile([128, 512], F32, tag="s", name="S2")
                            nc.tensor.matmul(
                                S1[:, vc:512],
                                kTd[p][0:64, ksl],
                                qT_sb[p][0:64, q0 + vc:q0 + 512],
                                start=True, stop=True,
                            )
                            nc.tensor.matmul(
                                S2[:, vc:512],
                                kTd[p][64:128, ksl],
                                qT_sb[p][64:128, q0 + vc:q0 + 512],
                                start=True, stop=True,
                            )
                            ET1 = etp.tile([128, 512], F32R, tag="e1", name="ET1")
                            ET2 = etp.tile([128, 512], F32R, tag="e2", name="ET2")
                            nc.scalar.activation(ET1[:, vc:512], S1[:, vc:512], Act.Exp)
                            nc.scalar.activation(ET2[:, vc:512], S2[:, vc:512], Act.Exp)
                            if j >= 0:  # diagonal tile: mask mixed 128x128 block
                                msl = slice(vc, vc + 128)
                                nc.vector.tensor_mul(ET1[:, msl], ET1[:, msl], tri[:])
                                nc.vector.tensor_mul(ET2[:, msl], ET2[:, msl], tri[:])
                            vt = v_sb[ki][:, vh * 128:vh * 128 + 128]
                            nc.tensor.matmul(OT1[:, vc:512], vt, ET1[:, vc:512],
                                             start=st, stop=sp_)
                            nc.tensor.matmul(R1[:, vc:512], ones[:], ET1[:, vc:512],
                                             start=st, stop=sp_)
                            nc.tensor.matmul(OT2[:, vc:512], vt, ET2[:, vc:512],
                                             start=st, stop=sp_)
                            nc.tensor.matmul(R2[:, vc:512], ones[:], ET2[:, vc:512],
                                             start=st, stop=sp_)

                        # epilogue: normalize + differential combine + RMS
                        rcp2 = ebp.tile([128, 512], F32, tag="eb", name="rcp2")
                        nc.vector.reciprocal(rcp2[:], R2[:])
                        mb = ebp.tile([128, 512], F32, tag="eb", name="mb")
                        nc.vector.scalar_tensor_tensor(
                            mb[:], R1[:], lam[:, 0:1], rcp2[:], Alu.mult, Alu.mult)
                        r1e = ebp.tile([128, 512], F32, tag="eb", name="r1e")
                        nc.vector.tensor_scalar_mul(r1e[:], R1[:], math.sqrt(EPS))
                        tt = ebp.tile([128, 512], F32, tag="eb", name="tt")
                        nc.vector.tensor_mul(tt[:], OT2[:], mb[:])
                        u = ebp.tile([128, 512], F32, tag="eb", name="u")
                        nc.vector.tensor_sub(u[:], OT1[:], tt[:])
                        sq = ebp.tile([128, 512], F32R, tag="eb", name="sq")
                        nc.vector.tensor_mul(sq[:], u[:], u[:])
                        varp = psAcc.tile([128, 512], F32, tag="acc", name="varp")
                        nc.tensor.matmul(varp[:], ones[:], sq[:], start=True, stop=True)
                        t2 = ebp.tile([128, 512], F32, tag="eb", name="t2")
                        nc.vector.tensor_mul(t2[:], r1e[:], r1e[:])
                        pre = ebp.tile([128, 512], F32, tag="eb", name="pre")
                        nc.vector.scalar_tensor_tensor(
                            pre[:], varp[:], 1.0 / 128.0, t2[:], Alu.mult, Alu.add)
                        lnp = ebp.tile([128, 512], F32, tag="eb", name="lnp")
                        nc.scalar.activation(lnp[:], pre[:], Act.Ln)
                        sf = ebp.tile([128, 512], F32, tag="eb", name="sf")
                        nc.scalar.activation(sf[:], lnp[:], Act.Exp, scale=-0.5)
                        nc.vector.tensor_mul(otf[p][:, q0:q0 + 512], u[:], sf[:])

              # ---------------- Phase C: o_proj (row-parallel partial) -------
              with tc.tile_pool(name="wop", bufs=1) as wop, \
                 tc.tile_pool(name="outp", bufs=2) as outp, \
                 tc.tile_pool(name="psC", bufs=4, space="PSUM") as psC:
                wo_t = {}
                for n in range(4):
                    t = wop.tile([128, 2048], F32R, tag=f"wo{n}", name=f"wo{n}")
                    nc.sync.dma_start(
                        out=t[:].rearrange("p (four n) -> p four n", four=4),
                        in_=woT[:, n * 512:n * 512 + 512]
                            .rearrange("(four p) n -> p four n", four=4),
                    )
                    for kc in range(4):
                        wo_t[n, kc] = t[:, kc * 512:kc * 512 + 512]
                for m in range(16):
                    osb = outp.tile([128, 2048], F32, tag="ob", name="osb")
                    for n in range(4):
                        ps = psC.tile([128, 512], F32, tag="pc", name="pc")
                        for kc in range(4):
                            nc.tensor.matmul(
                                ps[:],
                                otf[kc][:, m * 128:m * 128 + 128],
                                wo_t[n, kc],
                                start=(kc == 0), stop=(kc == 3),
                            )
                        nc.vector.tensor_copy(osb[:, n * 512:n * 512 + 512], ps[:])
                    nc.sync.dma_start(out=out_d[m * 128:m * 128 + 128, :], in_=osb[:])

    nc.compile()
    return nc


def _host_tables():
    inv = ROPE_THETA ** (-np.arange(Dh, dtype=np.float64) / Dh)
    pos = np.arange(S, dtype=np.float64)
    fr = pos[:, None] * inv[None, :]              # [S, 64]
    cos = np.cos(fr).astype(np.float32)           # [S, 64]
    sin = np.sin(fr).astype(np.float32)
    d = np.arange(128) % 64
    cosT = cos[:, d].T.copy()                     # [128, S]
    sgn = np.where((np.arange(128) % 64) < 32, -1.0, 1.0).astype(np.float32)
    ssinT = (sin[:, d].T * sgn[:, None]).copy()
    tri = np.triu(np.ones((128, 128), np.float32))  # tri[k, q] = 1 if q >= k
    ones = np.ones((128, 128), np.float32)
    return np.ascontiguousarray(cosT), np.ascontiguousarray(ssinT), tri, ones


def kernel(hidden_states, Wq, Wk, Wv, Wo,
           lambda_q1, lambda_k1, lambda_q2, lambda_k2, subln_weight):
    from concourse.bass_utils import run_bass_kernel_spmd

    if "nc" not in _CACHE:
        _CACHE["nc"] = _build_nc()
        _CACHE["tables"] = _host_tables()
    nc = _CACHE["nc"]
    cosT, ssinT, tri, ones = _CACHE["tables"]

    f32 = np.float32
    hs = np.asarray(hidden_states, f32)
    Wq = np.asarray(Wq, f32)
    Wk = np.asarray(Wk, f32)
    Wv = np.asarray(Wv, f32)
    Wo = np.asarray(Wo, f32)
    subln = np.asarray(subln_weight, f32)

    lam1 = np.exp(np.sum(np.asarray(lambda_q1, f32) * np.asarray(lambda_k1, f32),
                         dtype=f32))
    lam2 = np.exp(np.sum(np.asarray(lambda_q2, f32) * np.asarray(lambda_k2, f32),
                         dtype=f32))
    lam_full = f32(lam1 - lam2 + LAMBDA_INIT)
    lam_arr = np.full((128, 1), lam_full, f32)

    scale = f32(Dh ** -0.5)
    wprime = (np.tile(subln, H) * f32(1.0 - LAMBDA_INIT)).astype(f32)  # [2048]
    WoS = Wo * wprime[None, :]

    in_maps = []
    for c in range(NCORES):
        b, r = c // TP, c % TP
        in_maps.append({
            "xT": np.ascontiguousarray(hs[b].T),
            "wqT": np.ascontiguousarray((Wq[512 * r:512 * r + 512, :] * scale).T),
            "wkT": np.ascontiguousarray(Wk[256 * r:256 * r + 256, :].T),
            "wvT": np.ascontiguousarray(Wv[256 * r:256 * r + 256, :].T),
            "woT": np.ascontiguousarray(WoS[:, 512 * r:512 * r + 512].T),
            "cosT": cosT, "ssinT": ssinT, "tri": tri, "ones": ones,
            "lam": lam_arr,
        })

    res = run_bass_kernel_spmd(nc, in_maps, core_ids=list(range(NCORES)))
    out = np.zeros((B, S, D), f32)
    for c in range(NCORES):
        out[c // TP] += res.results[c]["out"]
    return out



# revision 32
# speedup vs baseline: 1.2132x; 1.2132x over previous
"""DifferentialAttention (B=2, S=2048, D=2048, H=16, KVH=8) on 8 TRN2 NeuronCores.

Sharding: 8 cores = 2 (batch) x 4 (tensor-parallel head groups).
Core c = 4*b + r handles batch b and real heads 4r..4r+3:
  - column-parallel q/k/v projections (q heads 8r..8r+7, k heads 4r..4r+3,
    v heads 2r..2r+1), full causal differential attention for those heads,
  - row-parallel partial o_proj; host sums the 4 partials per batch.

Device design (per core), f32 PSUM accumulation everywhere:
  - host passes x^T and W^T slices in bf16 (halves phase-A HBM traffic);
    x/wq DMA issue interleaved so the first projection matmul starts ~3us in;
    wq is SBUF-resident (loaded once, not re-streamed per quarter)
  - scores computed transposed  S^T[k,q] = k . q  so exp -> AV needs no
    on-chip transposes; on-chip tensors stay f32/f32r for precision
  - softmax without max subtraction (scores ~ N(0,1)); row sums and their
    partition broadcast fused into ones-matmuls (R1/R2)
  - attention inner loop software-pipelined: score matmuls for k-tiles
    t..t+2 are emitted before the accumulate matmuls of tile t so the
    in-order PE queue never blocks on the PSUM accumulator handoff; OT1/OT2
    are copied out of PSUM on the gpsimd engine to free banks early
  - RMS-norm folded:  out = u * rsqrt(mean_d(u^2) + eps*r1^2)  with
    u = O1 - (lam*r1/r2)*O2,  O = E@v unnormalized,  r = rowsum(E);
    subln weight and (1 - lambda_init) folded into Wo on the host;
    rsqrt as exp(-0.5*ln(x)) with all Ln/Exp pairs batched after the last
    attention exp (ACT table loads: ~4 instead of ~33)
  - causal masking: matmuls sliced to valid columns (last diagonal tile
    padded to 256 cols to stay on the fast fp32r path); 0/1 masks applied
    on the gpsimd engine
  - kTd duplicate writes ride the gpsimd DGE queue and phase-C PSUM->SBUF
    copies the gpsimd engine, keeping SP free for prefetch and DVE for
    rope/epilogue
"""

import math
import numpy as np

B, S, D = 2, 2048, 2048
H, KVH = 16, 8
Dh = 64
TP = 4
NCORES = 8
LAYER_IDX = 2
LAMBDA_INIT = 0.8 - 0.6 * math.exp(-0.3 * LAYER_IDX)
EPS = 1e-5
ROPE_THETA = 10000.0

_CACHE = {}


def _build_nc():
    import concourse.bass as bass  # noqa: F401
    import concourse.tile as tile
    from concourse import bacc, mybir

    F32 = mybir.dt.float32
    F32R = mybir.dt.float32r
    BF16 = mybir.dt.bfloat16
    Act = mybir.ActivationFunctionType
    Alu = mybir.AluOpType

    nc = bacc.Bacc("TRN2", target_bir_lowering=False, debug=False)

    xT = nc.dram_tensor("xT", [D, S], BF16, kind="ExternalInput")
    wqT = nc.dram_tensor("wqT", [D, 512], BF16, kind="ExternalInput")
    wkT = nc.dram_tensor("wkT", [D, 256], BF16, kind="ExternalInput")
    wvT = nc.dram_tensor("wvT", [D, 256], BF16, kind="ExternalInput")
    woT = nc.dram_tensor("woT", [512, D], BF16, kind="ExternalInput")
    cosT_d = nc.dram_tensor("cosT", [128, S], F32, kind="ExternalInput")
    ssinT_d = nc.dram_tensor("ssinT", [128, S], F32, kind="ExternalInput")
    tri_d = nc.dram_tensor("tri", [128, 128], F32R, kind="ExternalInput")
    trip_d = nc.dram_tensor("trip", [128, 256], F32R, kind="ExternalInput")
    ones_d = nc.dram_tensor("ones", [128, 128], F32R, kind="ExternalInput")
    lam_d = nc.dram_tensor("lam", [128, 1], F32, kind="ExternalInput")
    out_d = nc.dram_tensor("out", [S, D], F32, kind="ExternalOutput")

    KD = D // 128  # 16 contraction tiles

    with tile.TileContext(nc) as tc:
        with tc.tile_pool(name="const", bufs=1) as constp, \
             tc.tile_pool(name="persist", bufs=1) as persist:

            cosT = constp.tile([128, S], F32, tag="cos")
            ssinT = constp.tile([128, S], F32, tag="ssin")
            tri = constp.tile([128, 128], F32R, tag="tri")
            trip = constp.tile([128, 256], F32R, tag="trip")
            ones = constp.tile([128, 128], F32R, tag="ones")
            lam = constp.tile([128, 1], F32, tag="lam")

            # per-(head-block, quarter) tiles so phase B's dependencies are
            # exactly the quarter it reads, not the whole projection
            qT_sb = {(m, sh): persist.tile([128, 512], F32R, tag=f"qT{m}_{sh}",
                                           name=f"qT{m}_{sh}")
                     for m in range(4) for sh in range(4)}
            kTd = {(h, sh): persist.tile([128, 512], F32R, tag=f"kTd{h}_{sh}",
                                         name=f"kTd{h}_{sh}")
                   for h in range(4) for sh in range(4)}
            v_sb = [persist.tile([128, 256], F32R, tag=f"v{ms}", name=f"v{ms}")
                    for ms in range(16)]

            # ---------------- Phase A: projections + RoPE ----------------
            # ropet opens first: its SBUF range is the last to drain at the
            # A->B boundary, so it must alias phase B's LATE-touched tiles
            # (uq/pre), not the ET pool that B needs immediately
            with tc.tile_pool(name="ropet", bufs=2) as rp, \
                 tc.tile_pool(name="xtp", bufs=8) as xtp, \
                 tc.tile_pool(name="wres", bufs=1) as wres, \
                 tc.tile_pool(name="psA", bufs=8, space="PSUM") as psA:

                # interleave the sh=0 x tiles with the resident wq tiles so
                # the first q matmul's operands arrive first
                wq_r = [wres.tile([128, 2048], BF16, tag=f"wqr{kp}",
                                  name=f"wqr{kp}") for kp in range(4)]
                xt_t = {}
                for kp in range(4):
                    xt_t[0, kp] = xtp.tile([128, 2048], BF16, tag="xt",
                                           name=f"xt0_{kp}")
                    # first tile pair split in half so the very first matmul's
                    # operands land in ~1.5us instead of ~3us
                    nhalf = 2 if kp == 0 else 1
                    for h in range(nhalf):
                        w = 2048 // nhalf
                        nc.sync.dma_start(
                            out=xt_t[0, kp][:, h * w:(h + 1) * w]
                                .rearrange("p (f n) -> p f n", n=512),
                            in_=xT[kp * 512 + h * (512 // nhalf):
                                   kp * 512 + (h + 1) * (512 // nhalf), 0:512]
                                .rearrange("(f p) n -> p f n", p=128),
                        )
                        nc.sync.dma_start(
                            out=wq_r[kp][:, h * w:(h + 1) * w]
                                .rearrange("p (f n) -> p f n", n=512),
                            in_=wqT[kp * 512 + h * (512 // nhalf):
                                    kp * 512 + (h + 1) * (512 // nhalf), :]
                                .rearrange("(f p) n -> p f n", p=128),
                        )
                wk_r = []
                wv_r = []

                def load_wkv_tables():
                    for kp in range(2):
                        t = wres.tile([128, 2048], BF16, tag=f"wkr{kp}",
                                      name=f"wkr{kp}")
                        nc.sync.dma_start(
                            out=t[:].rearrange("p (eight n) -> p eight n",
                                               eight=8),
                            in_=wkT[kp * 1024:kp * 1024 + 1024, :]
                                .rearrange("(eight p) n -> p eight n", eight=8),
                        )
                        wk_r.append(t)
                        t = wres.tile([128, 2048], BF16, tag=f"wvr{kp}",
                                      name=f"wvr{kp}")
                        nc.sync.dma_start(
                            out=t[:].rearrange("p (eight n) -> p eight n",
                                               eight=8),
                            in_=wvT[kp * 1024:kp * 1024 + 1024, :]
                                .rearrange("(eight p) n -> p eight n", eight=8),
                        )
                        wv_r.append(t)
                    nc.sync.dma_start(out=cosT[:], in_=cosT_d[:])
                    nc.sync.dma_start(out=ssinT[:], in_=ssinT_d[:])
                    nc.sync.dma_start(out=tri[:], in_=tri_d[:])
                    nc.sync.dma_start(out=trip[:], in_=trip_d[:])
                    nc.sync.dma_start(out=ones[:], in_=ones_d[:])
                    nc.sync.dma_start(out=lam[:], in_=lam_d[:])

                def wq_lhsT(kd, m):
                    return wq_r[kd // 4][:, (kd % 4) * 512 + m * 128:
                                         (kd % 4) * 512 + m * 128 + 128]

                def wk_lhsT(kd, m):
                    return wk_r[kd // 8][:, (kd % 8) * 256 + m * 128:
                                         (kd % 8) * 256 + m * 128 + 128]

                def wv_rhs(kd):
                    return wv_r[kd // 8][:, (kd % 8) * 256:(kd % 8) * 256 + 256]

                def rope_core(ps, gc0):
                    """RoPE on a [128, 512] psum tile; returns (ra, rt) to add."""
                    gsl = slice(gc0, gc0 + 512)
                    qraw = rp.tile([128, 512], F32, tag="qraw", name="qraw")
                    nc.scalar.copy(qraw[:], ps[:])
                    qsw = rp.tile([128, 512], F32, tag="qsw", name="qsw")
                    for blk in range(4):
                        sb_ = (blk ^ 1) * 32
                        nc.vector.tensor_copy(
                            qsw[blk * 32:blk * 32 + 32, :], qraw[sb_:sb_ + 32, :])
                    nc.vector.tensor_mul(qraw[:], qraw[:], cosT[:, gsl])
                    nc.vector.tensor_mul(qsw[:], qsw[:], ssinT[:, gsl])
                    return qraw, qsw

                def rope_epilogue(ps, dst, gc0):
                    ra, rt = rope_core(ps, gc0)
                    nc.vector.tensor_add(dst[:], ra[:], rt[:])

                def rope_epilogue_kdup(ps, m, sh, gc0):
                    """RoPE then duplicate each 64-row head half into kTd[2m+e]."""
                    ra, rt = rope_core(ps, gc0)
                    ktmp = rp.tile([128, 512], F32R, tag="ktmp", name="ktmp")
                    nc.vector.tensor_add(ktmp[:], ra[:], rt[:])
                    for e in range(2):
                        srcv = ktmp[e * 64:e * 64 + 64, :]
                        nc.gpsimd.dma_start(out=kTd[2 * m + e, sh][0:64, :],
                                            in_=srcv)
                        nc.gpsimd.dma_start(out=kTd[2 * m + e, sh][64:128, :],
                                            in_=srcv)

                for sh in range(4):
                    c0 = 512 * sh
                    if sh > 0:
                        for kp in range(4):
                            xt_t[sh, kp] = xtp.tile([128, 2048], BF16, tag="xt",
                                                    name=f"xt{sh}_{kp}")
                            nc.sync.dma_start(
                                out=xt_t[sh, kp][:]
                                    .rearrange("p (four n) -> p four n", four=4),
                                in_=xT[kp * 512:kp * 512 + 512, c0:c0 + 512]
                                    .rearrange("(four p) n -> p four n", four=4),
                            )

                    def xt_rhs(kd):
                        return xt_t[sh, kd // 4][:, (kd % 4) * 512:
                                                 (kd % 4) * 512 + 512]

                    def emit_qproj(post_mm=None):
                        qps = {}
                        for kd in range(KD):
                            for m in range(4):
                                if kd == 0:
                                    qps[m] = psA.tile([128, 512], F32, tag="pa",
                                                      name=f"qps{m}")
                                nc.tensor.matmul(
                                    qps[m][:],
                                    wq_lhsT(kd, m),
                                    xt_rhs(kd),
                                    start=(kd == 0), stop=(kd == KD - 1),
                                )
                        if post_mm is not None:
                            # table/weight DMAs are emitted here -- after the
                            # q matmuls but BEFORE the ropes that read the
                            # tables (emission order defines dependencies)
                            post_mm()
                        for m in range(4):
                            rope_epilogue(qps[m], qT_sb[m, sh], c0)

                    def emit_kproj():
                        kps = {}
                        for kd in range(KD):
                            for m in range(2):
                                if kd == 0:
                                    kps[m] = psA.tile([128, 512], F32, tag="pa",
                                                      name=f"kps{m}")
                                nc.tensor.matmul(
                                    kps[m][:],
                                    wk_lhsT(kd, m),
                                    xt_rhs(kd),
                                    start=(kd == 0), stop=(kd == KD - 1),
                                )
                        for m in range(2):
                            rope_epilogue_kdup(kps[m], m, sh, c0)


                    def emit_vproj():
                        vps = {}
                        for kd in range(KD):
                            for ms in range(4):
                                if kd == 0:
                                    vps[ms] = psA.tile([128, 512], F32, tag="pa",
                                                       name=f"vps{ms}")
                                nc.tensor.matmul(
                                    vps[ms][:, 0:256],
                                    xt_rhs(kd)[:, ms * 128:ms * 128 + 128],
                                    wv_rhs(kd),
                                    start=(kd == 0), stop=(kd == KD - 1),
                                )
                        for ms in range(4):
                            nc.scalar.copy(v_sb[sh * 4 + ms][:],
                                           vps[ms][:, 0:256])

                    # on the last quarter run k first: its rope+dup chain
                    # feeds phase B and drains under the q/v matmuls
                    if sh == 3:
                        emit_kproj()
                        emit_qproj()
                        emit_vproj()
                    else:
                        emit_qproj(load_wkv_tables if sh == 0 else None)
                        emit_kproj()
                        emit_vproj()

            with tc.tile_pool(name="otfp", bufs=1) as otfp, \
                 tc.tile_pool(name="wop", bufs=1) as wop:
              # u_q[qi][:, p*512:(p+1)*512] holds head-block p's (unscaled,
              # later sf-scaled in place) attention output for q-block qi --
              # one tile per qi so phase C's m-blocks depend only on their
              # own qi group
              u_q = [otfp.tile([128, 2048], BF16, tag=f"uq{qi}",
                               name=f"uq{qi}") for qi in range(4)]
              pre_q = [otfp.tile([128, 2048], BF16, tag=f"preq{qi}",
                                 name=f"preq{qi}") for qi in range(4)]
              sfp = otfp  # sf scratch shares the long-lived pool scope

              def emit_sf_chain(qi):
                  """rsqrt(pre) and scale u in place: Ln -> Exp(-0.5*) -> mul.
                  The scale runs as two halves on DVE and gpsimd in parallel
                  to shorten the chain into phase C."""
                  sf = sfp.tile([128, 2048], F32, tag="sf", name=f"sf{qi}")
                  nc.scalar.activation(sf[:], pre_q[qi][:], Act.Ln)
                  nc.scalar.activation(sf[:], sf[:], Act.Exp, scale=-0.5)
                  nc.vector.tensor_mul(u_q[qi][:, 0:1024],
                                       u_q[qi][:, 0:1024], sf[:, 0:1024])
                  nc.gpsimd.tensor_mul(u_q[qi][:, 1024:2048],
                                       u_q[qi][:, 1024:2048], sf[:, 1024:2048])
              # prefetch o_proj weights while phase B runs
              wo_t = {}
              for n in range(4):
                t = wop.tile([128, 2048], BF16, tag=f"wo{n}", name=f"wo{n}")
                nc.sync.dma_start(
                    out=t[:].rearrange("p (four n) -> p four n", four=4),
                    in_=woT[:, n * 512:n * 512 + 512]
                        .rearrange("(four p) n -> p four n", four=4),
                )
                for kc in range(4):
                    wo_t[n, kc] = t[:, kc * 512:kc * 512 + 512]

              # ---------------- Phase B: attention ----------------
              # diag tile j -> first computed column within the q block
              DIAG_C0 = [0, 128, 256, 256]
              QI_ORDER = {0: (0, 1, 2, 3), 1: (3, 2, 1, 0),
                          2: (3, 2, 1, 0), 3: (0, 1, 2, 3)}
              SHIFT = 4  # accumulate matmuls trail the score matmuls
              with tc.tile_pool(name="etp", bufs=5) as etp, \
                 tc.tile_pool(name="ebp", bufs=7) as ebp, \
                 tc.tile_pool(name="psS", bufs=3, space="PSUM") as psS, \
                 tc.tile_pool(name="psAcc", bufs=4, space="PSUM") as psAcc, \
                 tc.tile_pool(name="psV", bufs=1, space="PSUM") as psV:

                for p in range(4):
                    vh = p // 2
                    for qi in QI_ORDER[p]:
                        q0 = 512 * qi
                        kis = list(range(4 * qi, 4 * qi + 4)) + list(range(4 * qi))
                        nt = len(kis)
                        OT1 = psAcc.tile([128, 512], F32, tag="acc", name="OT1")
                        OT2 = psAcc.tile([128, 512], F32, tag="acc", name="OT2")
                        R1 = psAcc.tile([128, 512], F32, tag="acc", name="R1")
                        R2 = psAcc.tile([128, 512], F32, tag="acc", name="R2")
                        ets = {}

                        def emit_front(t):
                            """scores + exp + mask for k-tile index t."""
                            ki = kis[t]
                            j = ki - 4 * qi
                            vc = DIAG_C0[j] if 0 <= j < 4 else 0
                            S1 = psS.tile([128, 512], F32, tag="s", name="S1")
                            S2 = psS.tile([128, 512], F32, tag="s", name="S2")
                            kt = kTd[p, ki // 4]
                            kcs = slice((ki % 4) * 128, (ki % 4) * 128 + 128)
                            qt = qT_sb[p, qi]
                            nc.tensor.matmul(
                                S1[:, vc:512],
                                kt[0:64, kcs],
                                qt[0:64, vc:512],
                                start=True, stop=True,
                            )
                            nc.tensor.matmul(
                                S2[:, vc:512],
                                kt[64:128, kcs],
                                qt[64:128, vc:512],
                                start=True, stop=True,
                            )
                            ET = etp.tile([128, 1024], F32R, tag="e", name="ET")
                            nc.scalar.activation(ET[:, vc:512], S1[:, vc:512],
                                                 Act.Exp)
                            nc.scalar.activation(ET[:, 512 + vc:1024],
                                                 S2[:, vc:512], Act.Exp)
                            if 0 <= j < 3:
                                msl = slice(vc, vc + 128)
                                nc.gpsimd.tensor_mul(ET[:, msl], ET[:, msl],
                                                     tri[:])
                                nc.gpsimd.tensor_mul(
                                    ET[:, 512 + vc:512 + vc + 128],
                                    ET[:, 512 + vc:512 + vc + 128], tri[:])
                            elif j == 3:
                                nc.gpsimd.tensor_mul(ET[:, 256:512],
                                                     ET[:, 256:512], trip[:])
                                nc.gpsimd.tensor_mul(ET[:, 768:1024],
                                                     ET[:, 768:1024], trip[:])
                            ets[t] = (ET, vc)

                        def emit_accum(t):
                            """AV + rowsum accumulation for k-tile index t."""
                            ki = kis[t]
                            ET, vc = ets.pop(t)
                            st = (t == 0)
                            sp_ = (t == nt - 1)
                            vt = v_sb[ki][:, vh * 128:vh * 128 + 128]
                            nc.tensor.matmul(OT1[:, vc:512], vt, ET[:, vc:512],
                                             start=st, stop=sp_)
                            nc.tensor.matmul(R1[:, vc:512], ones[:],
                                             ET[:, vc:512], start=st, stop=sp_)
                            nc.tensor.matmul(OT2[:, vc:512], vt,
                                             ET[:, 512 + vc:1024],
                                             start=st, stop=sp_)
                            nc.tensor.matmul(R2[:, vc:512], ones[:],
                                             ET[:, 512 + vc:1024],
                                             start=st, stop=sp_)

                        for t in range(nt):
                            emit_front(t)
                            if t >= SHIFT:
                                emit_accum(t - SHIFT)
                        for t in range(max(0, nt - SHIFT), nt):
                            emit_accum(t)

                        # epilogue: free the PSUM accumulators early (OT1/OT2
                        # copied out on gpsimd), differential combine + RMS
                        # stats; the rsqrt Ln/Exp is deferred past phase B.
                        ot2c = ebp.tile([128, 512], F32, tag="eb", name="ot2c")
                        nc.vector.tensor_copy(ot2c[:], OT2[:])
                        ot1c = ebp.tile([128, 512], F32, tag="eb", name="ot1c")
                        nc.scalar.copy(ot1c[:], OT1[:])
                        rcp2 = ebp.tile([128, 512], F32, tag="eb", name="rcp2")
                        nc.vector.reciprocal(rcp2[:], R2[:])
                        mb = ebp.tile([128, 512], F32, tag="eb", name="mb")
                        nc.vector.scalar_tensor_tensor(
                            mb[:], R1[:], lam[:, 0:1], rcp2[:], Alu.mult, Alu.mult)
                        r1e = ebp.tile([128, 512], F32, tag="eb", name="r1e")
                        nc.vector.tensor_scalar_mul(r1e[:], R1[:],
                                                    math.sqrt(EPS))
                        t2 = ebp.tile([128, 512], F32, tag="eb", name="t2")
                        nc.vector.tensor_mul(t2[:], r1e[:], r1e[:])
                        tt = ebp.tile([128, 512], F32, tag="eb", name="tt")
                        nc.vector.tensor_mul(tt[:], ot2c[:], mb[:])
                        u = u_q[qi][:, p * 512:p * 512 + 512]
                        nc.vector.tensor_sub(u, ot1c[:], tt[:])
                        sq = ebp.tile([128, 512], F32R, tag="eb", name="sq")
                        nc.vector.tensor_mul(sq[:], u, u)
                        varp = psV.tile([128, 512], F32, tag="v", name="varp")
                        nc.tensor.matmul(varp[:], ones[:], sq[:], start=True,
                                         stop=True)
                        nc.vector.scalar_tensor_tensor(
                            pre_q[qi][:, p * 512:p * 512 + 512],
                            varp[:], 1.0 / 128.0, t2[:], Alu.mult, Alu.add)


              # -------- Phase C: o_proj (row-parallel partial) --------
              with tc.tile_pool(name="outp", bufs=6) as outp, \
                 tc.tile_pool(name="psC", bufs=4, space="PSUM") as psC:
                # qi=0's chain gates phase C's first m-blocks; later chains
                # hide under phase C's own matmuls
                for qi in range(4):
                    emit_sf_chain(qi)

                for m in range(16):
                    qi, mq = m // 4, m % 4
                    for n in range(4):
                        ps = psC.tile([128, 512], F32, tag="pc", name="pc")
                        for kc in range(4):
                            nc.tensor.matmul(
                                ps[:],
                                u_q[qi][:, kc * 512 + mq * 128:
                                        kc * 512 + mq * 128 + 128],
                                wo_t[n, kc],
                                start=(kc == 0), stop=(kc == 3),
                            )
                        # alternate the PSUM->SBUF copy between ACT and DVE
                        # (either alone can't keep pace with the matmuls;
                        # gpsimd cannot read PSUM)
                        cpeng = None
                        last = (m == 15 and n == 3)
                        if not last:
                            ob = outp.tile([128, 512], F32, tag="ob", name="ob")
                            if (m * 4 + n) % 2 == 0:
                                nc.scalar.copy(ob[:], ps[:])
                            else:
                                nc.vector.tensor_copy(ob[:], ps[:])
                            nc.sync.dma_start(
                                out=out_d[m * 128:m * 128 + 128,
                                          n * 512:n * 512 + 512],
                                in_=ob[:])
                        else:
                            # final chunk: halves, so the post-matmul drain is
                            # one short copy+DMA
                            for h in range(2):
                                hs_ = slice(h * 256, h * 256 + 256)
                                ob = outp.tile([128, 512], F32, tag="ob",
                                               name="ob")
                                if h == 0:
                                    nc.scalar.copy(ob[:, 0:256], ps[:, hs_])
                                else:
                                    nc.vector.tensor_copy(ob[:, 0:256],
                                                          ps[:, hs_])
                                nc.sync.dma_start(
                                    out=out_d[m * 128:m * 128 + 128,
                                              n * 512 + h * 256:
                                              n * 512 + h * 256 + 256],
                                    in_=ob[:, 0:256])

    nc.compile()
    return nc


def _host_tables():
    inv = ROPE_THETA ** (-np.arange(Dh, dtype=np.float64) / Dh)
    pos = np.arange(S, dtype=np.float64)
    fr = pos[:, None] * inv[None, :]              # [S, 64]
    cos = np.cos(fr).astype(np.float32)           # [S, 64]
    sin = np.sin(fr).astype(np.float32)
    d = np.arange(128) % 64
    cosT = cos[:, d].T.copy()                     # [128, S]
    sgn = np.where((np.arange(128) % 64) < 32, -1.0, 1.0).astype(np.float32)
    ssinT = (sin[:, d].T * sgn[:, None]).copy()
    tri = np.triu(np.ones((128, 128), np.float32))  # tri[k, q] = 1 if q >= k
    trip = np.concatenate([np.zeros((128, 128), np.float32), tri], axis=1)
    ones = np.ones((128, 128), np.float32)
    return (np.ascontiguousarray(cosT), np.ascontiguousarray(ssinT),
            tri, np.ascontiguousarray(trip), ones)


def kernel(hidden_states, Wq, Wk, Wv, Wo,
           lambda_q1, lambda_k1, lambda_q2, lambda_k2, subln_weight):
    import ml_dtypes
    from concourse.bass_utils import run_bass_kernel_spmd

    if "nc" not in _CACHE:
        _CACHE["nc"] = _build_nc()
        _CACHE["tables"] = _host_tables()
    nc = _CACHE["nc"]
    cosT, ssinT, tri, trip, ones = _CACHE["tables"]

    f32 = np.float32
    bf16 = ml_dtypes.bfloat16
    hs = np.asarray(hidden_states, f32)
    Wq = np.asarray(Wq, f32)
    Wk = np.asarray(Wk, f32)
    Wv = np.asarray(Wv, f32)
    Wo = np.asarray(Wo, f32)
    subln = np.asarray(subln_weight, f32)

    lam1 = np.exp(np.sum(np.asarray(lambda_q1, f32) * np.asarray(lambda_k1, f32),
                         dtype=f32))
    lam2 = np.exp(np.sum(np.asarray(lambda_q2, f32) * np.asarray(lambda_k2, f32),
                         dtype=f32))
    lam_full = f32(lam1 - lam2 + LAMBDA_INIT)
    lam_arr = np.full((128, 1), lam_full, f32)

    scale = f32(Dh ** -0.5)
    wprime = (np.tile(subln, H) * f32(1.0 - LAMBDA_INIT)).astype(f32)  # [2048]
    WoS = Wo * wprime[None, :]

    in_maps = []
    for c in range(NCORES):
        b, r = c // TP, c % TP
        in_maps.append({
            "xT": np.ascontiguousarray(hs[b].T).astype(bf16),
            "wqT": np.ascontiguousarray(
                (Wq[512 * r:512 * r + 512, :] * scale).T).astype(bf16),
            "wkT": np.ascontiguousarray(
                Wk[256 * r:256 * r + 256, :].T).astype(bf16),
            "wvT": np.ascontiguousarray(
                Wv[256 * r:256 * r + 256, :].T).astype(bf16),
            "woT": np.ascontiguousarray(
                WoS[:, 512 * r:512 * r + 512].T).astype(bf16),
            "cosT": cosT, "ssinT": ssinT, "tri": tri, "trip": trip,
            "ones": ones, "lam": lam_arr,
        })

    res = run_bass_kernel_spmd(nc, in_maps, core_ids=list(range(NCORES)))
    out = np.zeros((B, S, D), f32)
    for c in range(NCORES):
        out[c // TP] += res.results[c]["out"]
    return out


# revision 42
# speedup vs baseline: 1.2285x; 1.0126x over previous
"""DifferentialAttention (B=2, S=2048, D=2048, H=16, KVH=8) on 8 TRN2 NeuronCores.

Sharding: 8 cores = 2 (batch) x 4 (tensor-parallel head groups).
Core c = 4*b + r handles batch b and real heads 4r..4r+3:
  - column-parallel q/k/v projections (q heads 8r..8r+7, k heads 4r..4r+3,
    v heads 2r..2r+1), full causal differential attention for those heads,
  - row-parallel partial o_proj; host sums the 4 partials per batch.

Device design (per core), f32 PSUM accumulation everywhere:
  - host passes x^T and W^T slices in bf16 (halves phase-A HBM traffic);
    x/wq DMA issue interleaved so the first projection matmul starts ~3us in;
    wq is SBUF-resident (loaded once, not re-streamed per quarter)
  - scores computed transposed  S^T[k,q] = k . q  so exp -> AV needs no
    on-chip transposes; on-chip tensors stay f32/f32r for precision
  - softmax without max subtraction (scores ~ N(0,1)); row sums and their
    partition broadcast fused into ones-matmuls (R1/R2)
  - attention inner loop software-pipelined: score matmuls run a few
    k-tiles ahead of the accumulate matmuls so the in-order PE queue never
    blocks on the PSUM accumulator handoff; OT1/OT2 are copied out of PSUM
    on the DVE to free banks early (gpsimd cannot touch PSUM)
  - RMS-norm folded:  out = u * rsqrt(mean_d(u^2) + eps*r1^2)  with
    u = O1 - (lam*r1/r2)*O2,  O = E@v unnormalized,  r = rowsum(E);
    subln weight and (1 - lambda_init) folded into Wo on the host;
    rsqrt as exp(-0.5*ln(x)) with the Ln/Exp pairs batched per q-quarter
    after the last attention exp (ACT table loads: ~9 instead of ~33);
    u stored bf16 per q-quarter so o_proj depends on exactly one quarter
  - causal masking: matmuls sliced to valid columns (last diagonal tile
    padded to 256 cols to stay on the fast fp32r path); 0/1 masks applied
    on the gpsimd engine (SBUF-only)
  - kTd duplicate writes ride the gpsimd DGE queue; rope scratch pool opens
    first so its late-draining SBUF range aliases phase B's late-touched
    tiles; p3 runs qi (2,3,1,0) and o_proj's m-groups follow the same
    order so the phase B tail drains under o_proj matmuls
"""

import math
import numpy as np

B, S, D = 2, 2048, 2048
H, KVH = 16, 8
Dh = 64
TP = 4
NCORES = 8
LAYER_IDX = 2
LAMBDA_INIT = 0.8 - 0.6 * math.exp(-0.3 * LAYER_IDX)
EPS = 1e-5
ROPE_THETA = 10000.0

_CACHE = {}


def _build_nc():
    import concourse.bass as bass  # noqa: F401
    import concourse.tile as tile
    from concourse import bacc, mybir

    F32 = mybir.dt.float32
    F32R = mybir.dt.float32r
    BF16 = mybir.dt.bfloat16
    Act = mybir.ActivationFunctionType
    Alu = mybir.AluOpType

    nc = bacc.Bacc("TRN2", target_bir_lowering=False, debug=False)

    xT = nc.dram_tensor("xT", [D, S], BF16, kind="ExternalInput")
    wqT = nc.dram_tensor("wqT", [D, 512], BF16, kind="ExternalInput")
    wkT = nc.dram_tensor("wkT", [D, 256], BF16, kind="ExternalInput")
    wvT = nc.dram_tensor("wvT", [D, 256], BF16, kind="ExternalInput")
    woT = nc.dram_tensor("woT", [512, D], BF16, kind="ExternalInput")
    cosT_d = nc.dram_tensor("cosT", [128, S], F32, kind="ExternalInput")
    ssinT_d = nc.dram_tensor("ssinT", [128, S], F32, kind="ExternalInput")
    tri_d = nc.dram_tensor("tri", [128, 128], F32R, kind="ExternalInput")
    trip_d = nc.dram_tensor("trip", [128, 256], F32R, kind="ExternalInput")
    ones_d = nc.dram_tensor("ones", [128, 128], F32R, kind="ExternalInput")
    lam_d = nc.dram_tensor("lam", [128, 1], F32, kind="ExternalInput")
    out_d = nc.dram_tensor("out", [S, D], F32, kind="ExternalOutput")

    KD = D // 128  # 16 contraction tiles

    with tile.TileContext(nc) as tc:
        with tc.tile_pool(name="const", bufs=1) as constp, \
             tc.tile_pool(name="persist", bufs=1) as persist:

            cosT = constp.tile([128, S], F32, tag="cos")
            ssinT = constp.tile([128, S], F32, tag="ssin")
            tri = constp.tile([128, 128], F32R, tag="tri")
            trip = constp.tile([128, 256], F32R, tag="trip")
            ones = constp.tile([128, 128], F32R, tag="ones")
            lam = constp.tile([128, 1], F32, tag="lam")

            # per-(head-block, quarter) tiles so phase B's dependencies are
            # exactly the quarter it reads, not the whole projection
            qT_sb = {(m, sh): persist.tile([128, 512], F32R, tag=f"qT{m}_{sh}",
                                           name=f"qT{m}_{sh}")
                     for m in range(4) for sh in range(4)}
            kTd = {(h, sh): persist.tile([128, 512], F32R, tag=f"kTd{h}_{sh}",
                                         name=f"kTd{h}_{sh}")
                   for h in range(4) for sh in range(4)}
            v_sb = [persist.tile([128, 256], F32R, tag=f"v{ms}", name=f"v{ms}")
                    for ms in range(16)]

            # ---------------- Phase A: projections + RoPE ----------------
            # ropet opens first: its SBUF range is the last to drain at the
            # A->B boundary, so it must alias phase B's LATE-touched tiles
            # (uq/pre), not the ET pool that B needs immediately
            with tc.tile_pool(name="ropet", bufs=2) as rp, \
                 tc.tile_pool(name="xtp", bufs=8) as xtp, \
                 tc.tile_pool(name="wres", bufs=1) as wres, \
                 tc.tile_pool(name="psA", bufs=8, space="PSUM") as psA:

                # interleave the sh=0 x tiles with the resident wq tiles so
                # the first q matmul's operands arrive first
                wq_r = [wres.tile([128, 2048], BF16, tag=f"wqr{kp}",
                                  name=f"wqr{kp}") for kp in range(4)]
                xt_t = {}
                for kp in range(4):
                    xt_t[0, kp] = xtp.tile([128, 2048], BF16, tag="xt",
                                           name=f"xt0_{kp}")
                    # first tile pair split in half so the very first matmul's
                    # operands land in ~1.5us instead of ~3us
                    nhalf = 2 if kp == 0 else 1
                    for h in range(nhalf):
                        w = 2048 // nhalf
                        nc.sync.dma_start(
                            out=xt_t[0, kp][:, h * w:(h + 1) * w]
                                .rearrange("p (f n) -> p f n", n=512),
                            in_=xT[kp * 512 + h * (512 // nhalf):
                                   kp * 512 + (h + 1) * (512 // nhalf), 0:512]
                                .rearrange("(f p) n -> p f n", p=128),
                        )
                        nc.sync.dma_start(
                            out=wq_r[kp][:, h * w:(h + 1) * w]
                                .rearrange("p (f n) -> p f n", n=512),
                            in_=wqT[kp * 512 + h * (512 // nhalf):
                                    kp * 512 + (h + 1) * (512 // nhalf), :]
                                .rearrange("(f p) n -> p f n", p=128),
                        )
                wk_r = []
                wv_r = []

                def load_wkv_tables():
                    for kp in range(2):
                        t = wres.tile([128, 2048], BF16, tag=f"wkr{kp}",
                                      name=f"wkr{kp}")
                        nc.sync.dma_start(
                            out=t[:].rearrange("p (eight n) -> p eight n",
                                               eight=8),
                            in_=wkT[kp * 1024:kp * 1024 + 1024, :]
                                .rearrange("(eight p) n -> p eight n", eight=8),
                        )
                        wk_r.append(t)
                        t = wres.tile([128, 2048], BF16, tag=f"wvr{kp}",
                                      name=f"wvr{kp}")
                        nc.sync.dma_start(
                            out=t[:].rearrange("p (eight n) -> p eight n",
                                               eight=8),
                            in_=wvT[kp * 1024:kp * 1024 + 1024, :]
                                .rearrange("(eight p) n -> p eight n", eight=8),
                        )
                        wv_r.append(t)
                    nc.sync.dma_start(out=cosT[:], in_=cosT_d[:])
                    nc.sync.dma_start(out=ssinT[:], in_=ssinT_d[:])
                    nc.sync.dma_start(out=tri[:], in_=tri_d[:])
                    nc.sync.dma_start(out=trip[:], in_=trip_d[:])
                    nc.sync.dma_start(out=ones[:], in_=ones_d[:])
                    nc.sync.dma_start(out=lam[:], in_=lam_d[:])

                def wq_lhsT(kd, m):
                    return wq_r[kd // 4][:, (kd % 4) * 512 + m * 128:
                                         (kd % 4) * 512 + m * 128 + 128]

                def wk_lhsT(kd, m):
                    return wk_r[kd // 8][:, (kd % 8) * 256 + m * 128:
                                         (kd % 8) * 256 + m * 128 + 128]

                def wv_rhs(kd):
                    return wv_r[kd // 8][:, (kd % 8) * 256:(kd % 8) * 256 + 256]

                def rope_core(ps, gc0):
                    """RoPE on a [128, 512] psum tile; returns (ra, rt) to add."""
                    gsl = slice(gc0, gc0 + 512)
                    qraw = rp.tile([128, 512], F32, tag="qraw", name="qraw")
                    nc.scalar.copy(qraw[:], ps[:])
                    qsw = rp.tile([128, 512], F32, tag="qsw", name="qsw")
                    for blk in range(4):
                        sb_ = (blk ^ 1) * 32
                        nc.vector.tensor_copy(
                            qsw[blk * 32:blk * 32 + 32, :], qraw[sb_:sb_ + 32, :])
                    nc.vector.tensor_mul(qraw[:], qraw[:], cosT[:, gsl])
                    nc.vector.tensor_mul(qsw[:], qsw[:], ssinT[:, gsl])
                    return qraw, qsw

                def rope_epilogue(ps, dst, gc0):
                    ra, rt = rope_core(ps, gc0)
                    nc.vector.tensor_add(dst[:], ra[:], rt[:])

                def rope_epilogue_kdup(ps, m, sh, gc0):
                    """RoPE then duplicate each 64-row head half into kTd[2m+e]."""
                    ra, rt = rope_core(ps, gc0)
                    ktmp = rp.tile([128, 512], F32R, tag="ktmp", name="ktmp")
                    nc.vector.tensor_add(ktmp[:], ra[:], rt[:])
                    for e in range(2):
                        srcv = ktmp[e * 64:e * 64 + 64, :]
                        nc.gpsimd.dma_start(out=kTd[2 * m + e, sh][0:64, :],
                                            in_=srcv)
                        nc.gpsimd.dma_start(out=kTd[2 * m + e, sh][64:128, :],
                                            in_=srcv)

                for sh in range(4):
                    c0 = 512 * sh
                    if sh > 0:
                        for kp in range(4):
                            xt_t[sh, kp] = xtp.tile([128, 2048], BF16, tag="xt",
                                                    name=f"xt{sh}_{kp}")
                            nc.sync.dma_start(
                                out=xt_t[sh, kp][:]
                                    .rearrange("p (four n) -> p four n", four=4),
                                in_=xT[kp * 512:kp * 512 + 512, c0:c0 + 512]
                                    .rearrange("(four p) n -> p four n", four=4),
                            )

                    def xt_rhs(kd):
                        return xt_t[sh, kd // 4][:, (kd % 4) * 512:
                                                 (kd % 4) * 512 + 512]

                    def emit_qproj(post_mm=None):
                        qps = {}
                        for kd in range(KD):
                            for m in range(4):
                                if kd == 0:
                                    qps[m] = psA.tile([128, 512], F32, tag="pa",
                                                      name=f"qps{m}")
                                nc.tensor.matmul(
                                    qps[m][:],
                                    wq_lhsT(kd, m),
                                    xt_rhs(kd),
                                    start=(kd == 0), stop=(kd == KD - 1),
                                )
                        if post_mm is not None:
                            # table/weight DMAs are emitted here -- after the
                            # q matmuls but BEFORE the ropes that read the
                            # tables (emission order defines dependencies)
                            post_mm()
                        for m in range(4):
                            rope_epilogue(qps[m], qT_sb[m, sh], c0)

                    def emit_kproj():
                        kps = {}
                        for kd in range(KD):
                            for m in range(2):
                                if kd == 0:
                                    kps[m] = psA.tile([128, 512], F32, tag="pa",
                                                      name=f"kps{m}")
                                nc.tensor.matmul(
                                    kps[m][:],
                                    wk_lhsT(kd, m),
                                    xt_rhs(kd),
                                    start=(kd == 0), stop=(kd == KD - 1),
                                )
                        for m in range(2):
                            rope_epilogue_kdup(kps[m], m, sh, c0)


                    def emit_vproj():
                        vps = {}
                        for kd in range(KD):
                            for ms in range(4):
                                if kd == 0:
                                    vps[ms] = psA.tile([128, 512], F32, tag="pa",
                                                       name=f"vps{ms}")
                                nc.tensor.matmul(
                                    vps[ms][:, 0:256],
                                    xt_rhs(kd)[:, ms * 128:ms * 128 + 128],
                                    wv_rhs(kd),
                                    start=(kd == 0), stop=(kd == KD - 1),
                                )
                        for ms in range(4):
                            nc.scalar.copy(v_sb[sh * 4 + ms][:],
                                           vps[ms][:, 0:256])

                    # on the last quarter run k first: its rope+dup chain
                    # feeds phase B and drains under the q/v matmuls
                    if sh == 3:
                        emit_kproj()
                        emit_qproj()
                        emit_vproj()
                    else:
                        emit_qproj(load_wkv_tables if sh == 0 else None)
                        emit_kproj()
                        emit_vproj()

            with tc.tile_pool(name="otfp", bufs=1) as otfp, \
                 tc.tile_pool(name="wop", bufs=1) as wop:
              # u_q[qi][:, p*512:(p+1)*512] holds head-block p's (unscaled,
              # later sf-scaled in place) attention output for q-block qi --
              # one tile per qi so phase C's m-blocks depend only on their
              # own qi group
              u_q = [otfp.tile([128, 2048], BF16, tag=f"uq{qi}",
                               name=f"uq{qi}") for qi in range(4)]
              pre_q = [otfp.tile([128, 2048], BF16, tag=f"preq{qi}",
                                 name=f"preq{qi}") for qi in range(4)]
              sfp = otfp  # sf scratch shares the long-lived pool scope

              def emit_sf_chain(qi, eng="pool"):
                  """rsqrt(pre) and scale u in place: Ln -> Exp(-0.5*) -> mul.
                  The first chain's scale runs split DVE/gpsimd (latency);
                  later chains go to gpsimd so DVE can drain phase C's
                  PSUM->SBUF copies."""
                  sf = sfp.tile([128, 2048], F32, tag="sf", name=f"sf{qi}")
                  nc.scalar.activation(sf[:], pre_q[qi][:], Act.Ln)
                  nc.scalar.activation(sf[:], sf[:], Act.Exp, scale=-0.5)
                  if eng == "split":
                      nc.vector.tensor_mul(u_q[qi][:, 0:1024],
                                           u_q[qi][:, 0:1024], sf[:, 0:1024])
                      nc.gpsimd.tensor_mul(u_q[qi][:, 1024:2048],
                                           u_q[qi][:, 1024:2048],
                                           sf[:, 1024:2048])
                  else:
                      nc.gpsimd.tensor_mul(u_q[qi][:], u_q[qi][:], sf[:])
              # prefetch o_proj weights while phase B runs
              wo_t = {}
              for n in range(4):
                t = wop.tile([128, 2048], BF16, tag=f"wo{n}", name=f"wo{n}")
                nc.sync.dma_start(
                    out=t[:].rearrange("p (four n) -> p four n", four=4),
                    in_=woT[:, n * 512:n * 512 + 512]
                        .rearrange("(four p) n -> p four n", four=4),
                )
                for kc in range(4):
                    wo_t[n, kc] = t[:, kc * 512:kc * 512 + 512]

              # ---------------- Phase B: attention ----------------
              # diag tile j -> first computed column within the q block
              DIAG_C0 = [0, 128, 256, 256]
              # p3 runs qi (2,3,1,0): phase C's m-groups follow the same
              # readiness order, so the last (thin) block's epilogue and the
              # remaining sf chains drain under phase C's first matmuls
              QI_ORDER = {0: (0, 1, 2, 3), 1: (3, 2, 1, 0),
                          2: (3, 2, 1, 0), 3: (2, 3, 1, 0)}
              SHIFT = 3  # accumulate matmuls trail the score matmuls
              with tc.tile_pool(name="etp", bufs=6) as etp, \
                 tc.tile_pool(name="ebp", bufs=7) as ebp, \
                 tc.tile_pool(name="psS", bufs=3, space="PSUM") as psS, \
                 tc.tile_pool(name="psAcc", bufs=4, space="PSUM") as psAcc, \
                 tc.tile_pool(name="psV", bufs=1, space="PSUM") as psV:

                for p in range(4):
                    vh = p // 2
                    for qi in QI_ORDER[p]:
                        q0 = 512 * qi
                        kis = list(range(4 * qi, 4 * qi + 4)) + list(range(4 * qi))
                        nt = len(kis)
                        OT1 = psAcc.tile([128, 512], F32, tag="acc", name="OT1")
                        OT2 = psAcc.tile([128, 512], F32, tag="acc", name="OT2")
                        R1 = psAcc.tile([128, 512], F32, tag="acc", name="R1")
                        R2 = psAcc.tile([128, 512], F32, tag="acc", name="R2")
                        ets = {}

                        def emit_front(t):
                            """scores + exp + mask for k-tile index t."""
                            ki = kis[t]
                            j = ki - 4 * qi
                            vc = DIAG_C0[j] if 0 <= j < 4 else 0
                            S1 = psS.tile([128, 512], F32, tag="s", name="S1")
                            S2 = psS.tile([128, 512], F32, tag="s", name="S2")
                            kt = kTd[p, ki // 4]
                            kcs = slice((ki % 4) * 128, (ki % 4) * 128 + 128)
                            qt = qT_sb[p, qi]
                            nc.tensor.matmul(
                                S1[:, vc:512],
                                kt[0:64, kcs],
                                qt[0:64, vc:512],
                                start=True, stop=True,
                            )
                            nc.tensor.matmul(
                                S2[:, vc:512],
                                kt[64:128, kcs],
                                qt[64:128, vc:512],
                                start=True, stop=True,
                            )
                            ET = etp.tile([128, 1024], F32R, tag="e", name="ET")
                            nc.scalar.activation(ET[:, vc:512], S1[:, vc:512],
                                                 Act.Exp)
                            nc.scalar.activation(ET[:, 512 + vc:1024],
                                                 S2[:, vc:512], Act.Exp)
                            if 0 <= j < 3:
                                msl = slice(vc, vc + 128)
                                nc.gpsimd.tensor_mul(ET[:, msl], ET[:, msl],
                                                     tri[:])
                                nc.gpsimd.tensor_mul(
                                    ET[:, 512 + vc:512 + vc + 128],
                                    ET[:, 512 + vc:512 + vc + 128], tri[:])
                            elif j == 3:
                                nc.gpsimd.tensor_mul(ET[:, 256:512],
                                                     ET[:, 256:512], trip[:])
                                nc.gpsimd.tensor_mul(ET[:, 768:1024],
                                                     ET[:, 768:1024], trip[:])
                            ets[t] = (ET, vc)

                        def emit_accum(t):
                            """AV + rowsum accumulation for k-tile index t."""
                            ki = kis[t]
                            ET, vc = ets.pop(t)
                            st = (t == 0)
                            sp_ = (t == nt - 1)
                            vt = v_sb[ki][:, vh * 128:vh * 128 + 128]
                            nc.tensor.matmul(OT1[:, vc:512], vt, ET[:, vc:512],
                                             start=st, stop=sp_)
                            nc.tensor.matmul(R1[:, vc:512], ones[:],
                                             ET[:, vc:512], start=st, stop=sp_)
                            nc.tensor.matmul(OT2[:, vc:512], vt,
                                             ET[:, 512 + vc:1024],
                                             start=st, stop=sp_)
                            nc.tensor.matmul(R2[:, vc:512], ones[:],
                                             ET[:, 512 + vc:1024],
                                             start=st, stop=sp_)

                        for t in range(nt):
                            emit_front(t)
                            if t >= SHIFT:
                                emit_accum(t - SHIFT)
                        for t in range(max(0, nt - SHIFT), nt):
                            emit_accum(t)

                        if (p, qi) == (3, 0):
                            # last block: emit the first sf chain before this
                            # epilogue so its DVE scale precedes the epilogue
                            # in the DVE queue -- phase C's first m-group
                            # unblocks ~2.5us earlier
                            emit_sf_chain(2, "split")

                        # epilogue: free the PSUM accumulators early (OT1/OT2
                        # copied out on gpsimd), differential combine + RMS
                        # stats; the rsqrt Ln/Exp is deferred past phase B.
                        ot2c = ebp.tile([128, 512], F32, tag="eb", name="ot2c")
                        nc.vector.tensor_copy(ot2c[:], OT2[:])
                        ot1c = ebp.tile([128, 512], F32, tag="eb", name="ot1c")
                        nc.vector.tensor_copy(ot1c[:], OT1[:])
                        rcp2 = ebp.tile([128, 512], F32, tag="eb", name="rcp2")
                        nc.vector.reciprocal(rcp2[:], R2[:])
                        mb = ebp.tile([128, 512], F32, tag="eb", name="mb")
                        nc.vector.scalar_tensor_tensor(
                            mb[:], R1[:], lam[:, 0:1], rcp2[:], Alu.mult, Alu.mult)
                        r1e = ebp.tile([128, 512], F32, tag="eb", name="r1e")
                        nc.vector.tensor_scalar_mul(r1e[:], R1[:],
                                                    math.sqrt(EPS))
                        t2 = ebp.tile([128, 512], F32, tag="eb", name="t2")
                        nc.vector.tensor_mul(t2[:], r1e[:], r1e[:])
                        tt = ebp.tile([128, 512], F32, tag="eb", name="tt")
                        nc.vector.tensor_mul(tt[:], ot2c[:], mb[:])
                        u = u_q[qi][:, p * 512:p * 512 + 512]
                        nc.vector.tensor_sub(u, ot1c[:], tt[:])
                        sq = ebp.tile([128, 512], F32R, tag="eb", name="sq")
                        nc.vector.tensor_mul(sq[:], u, u)
                        varp = psV.tile([128, 512], F32, tag="v", name="varp")
                        nc.tensor.matmul(varp[:], ones[:], sq[:], start=True,
                                         stop=True)
                        nc.vector.scalar_tensor_tensor(
                            pre_q[qi][:, p * 512:p * 512 + 512],
                            varp[:], 1.0 / 128.0, t2[:], Alu.mult, Alu.add)


              # -------- Phase C: o_proj (row-parallel partial) --------
              with tc.tile_pool(name="outp", bufs=6) as outp, \
                 tc.tile_pool(name="psC", bufs=4, space="PSUM") as psC:
                # chain(2) was emitted at the end of phase B; the rest hide
                # under phase C's own matmuls
                for qi in (3, 1, 0):
                    emit_sf_chain(qi, "pool")

                for qi in (2, 3, 1, 0):
                  for mq in range(4):
                    m = qi * 4 + mq
                    for n in range(4):
                        ps = psC.tile([128, 512], F32, tag="pc", name="pc")
                        for kc in range(4):
                            nc.tensor.matmul(
                                ps[:],
                                u_q[qi][:, kc * 512 + mq * 128:
                                        kc * 512 + mq * 128 + 128],
                                wo_t[n, kc],
                                start=(kc == 0), stop=(kc == 3),
                            )
                        # alternate the PSUM->SBUF copy between ACT and DVE
                        # (either alone can't keep pace with the matmuls;
                        # gpsimd cannot read PSUM)
                        cpeng = None
                        last = (m == 3 and n == 3)
                        if not last:
                            ob = outp.tile([128, 512], F32, tag="ob", name="ob")
                            # DVE while the sf chains occupy ACT; later
                            # chunks alternate with ACT
                            if qi in (2, 3) or (m * 4 + n) % 2 == 1:
                                nc.vector.tensor_copy(ob[:], ps[:])
                            else:
                                nc.scalar.copy(ob[:], ps[:])
                            nc.sync.dma_start(
                                out=out_d[m * 128:m * 128 + 128,
                                          n * 512:n * 512 + 512],
                                in_=ob[:])
                        else:
                            # final chunk: halves, so the post-matmul drain is
                            # one short copy+DMA
                            for h in range(2):
                                hs_ = slice(h * 256, h * 256 + 256)
                                ob = outp.tile([128, 512], F32, tag="ob",
                                               name="ob")
                                nc.vector.tensor_copy(ob[:, 0:256],
                                                      ps[:, hs_])
                                nc.sync.dma_start(
                                    out=out_d[m * 128:m * 128 + 128,
                                              n * 512 + h * 256:
                                              n * 512 + h * 256 + 256],
                                    in_=ob[:, 0:256])

    nc.compile()
    return nc


def _host_tables():
    inv = ROPE_THETA ** (-np.arange(Dh, dtype=np.float64) / Dh)
    pos = np.arange(S, dtype=np.float64)
    fr = pos[:, None] * inv[None, :]              # [S, 64]
    cos = np.cos(fr).astype(np.float32)           # [S, 64]
    sin = np.sin(fr).astype(np.float32)
    d = np.arange(128) % 64
    cosT = cos[:, d].T.copy()                     # [128, S]
    sgn = np.where((np.arange(128) % 64) < 32, -1.0, 1.0).astype(np.float32)
    ssinT = (sin[:, d].T * sgn[:, None]).copy()
    tri = np.triu(np.ones((128, 128), np.float32))  # tri[k, q] = 1 if q >= k
    trip = np.concatenate([np.zeros((128, 128), np.float32), tri], axis=1)
    ones = np.ones((128, 128), np.float32)
    return (np.ascontiguousarray(cosT), np.ascontiguousarray(ssinT),
            tri, np.ascontiguousarray(trip), ones)


def kernel(hidden_states, Wq, Wk, Wv, Wo,
           lambda_q1, lambda_k1, lambda_q2, lambda_k2, subln_weight):
    import ml_dtypes
    from concourse.bass_utils import run_bass_kernel_spmd

    if "nc" not in _CACHE:
        _CACHE["nc"] = _build_nc()
        _CACHE["tables"] = _host_tables()
    nc = _CACHE["nc"]
    cosT, ssinT, tri, trip, ones = _CACHE["tables"]

    f32 = np.float32
    bf16 = ml_dtypes.bfloat16
    hs = np.asarray(hidden_states, f32)
    Wq = np.asarray(Wq, f32)
    Wk = np.asarray(Wk, f32)
    Wv = np.asarray(Wv, f32)
    Wo = np.asarray(Wo, f32)
    subln = np.asarray(subln_weight, f32)

    lam1 = np.exp(np.sum(np.asarray(lambda_q1, f32) * np.asarray(lambda_k1, f32),
                         dtype=f32))
    lam2 = np.exp(np.sum(np.asarray(lambda_q2, f32) * np.asarray(lambda_k2, f32),
                         dtype=f32))
    lam_full = f32(lam1 - lam2 + LAMBDA_INIT)
    lam_arr = np.full((128, 1), lam_full, f32)

    scale = f32(Dh ** -0.5)
    wprime = (np.tile(subln, H) * f32(1.0 - LAMBDA_INIT)).astype(f32)  # [2048]
    WoS = Wo * wprime[None, :]

    in_maps = []
    for c in range(NCORES):
        b, r = c // TP, c % TP
        in_maps.append({
            "xT": np.ascontiguousarray(hs[b].T).astype(bf16),
            "wqT": np.ascontiguousarray(
                (Wq[512 * r:512 * r + 512, :] * scale).T).astype(bf16),
            "wkT": np.ascontiguousarray(
                Wk[256 * r:256 * r + 256, :].T).astype(bf16),
            "wvT": np.ascontiguousarray(
                Wv[256 * r:256 * r + 256, :].T).astype(bf16),
            "woT": np.ascontiguousarray(
                WoS[:, 512 * r:512 * r + 512].T).astype(bf16),
            "cosT": cosT, "ssinT": ssinT, "tri": tri, "trip": trip,
            "ones": ones, "lam": lam_arr,
        })

    res = run_bass_kernel_spmd(nc, in_maps, core_ids=list(range(NCORES)))
    out = np.zeros((B, S, D), f32)
    for c in range(NCORES):
        out[c // TP] += res.results[c]["out"]
    return out


# revision 43
# speedup vs baseline: 1.2388x; 1.0084x over previous
"""DifferentialAttention (B=2, S=2048, D=2048, H=16, KVH=8) on 8 TRN2 NeuronCores.

Sharding: 8 cores = 2 (batch) x 4 (tensor-parallel head groups).
Core c = 4*b + r handles batch b and real heads 4r..4r+3:
  - column-parallel q/k/v projections (q heads 8r..8r+7, k heads 4r..4r+3,
    v heads 2r..2r+1), full causal differential attention for those heads,
  - row-parallel partial o_proj; host sums the 4 partials per batch.

Device design (per core), f32 PSUM accumulation everywhere:
  - host passes x^T and W^T slices in bf16 (halves phase-A HBM traffic);
    x/wq DMA issue interleaved so the first projection matmul starts ~3us in;
    wq is SBUF-resident (loaded once, not re-streamed per quarter)
  - scores computed transposed  S^T[k,q] = k . q  so exp -> AV needs no
    on-chip transposes; on-chip tensors stay f32/f32r for precision
  - softmax without max subtraction (scores ~ N(0,1)); row sums and their
    partition broadcast fused into ones-matmuls (R1/R2)
  - attention inner loop software-pipelined: score matmuls run a few
    k-tiles ahead of the accumulate matmuls so the in-order PE queue never
    blocks on the PSUM accumulator handoff; OT1/OT2 are copied out of PSUM
    on the DVE to free banks early (gpsimd cannot touch PSUM)
  - RMS-norm folded:  out = u * rsqrt(mean_d(u^2) + eps*r1^2)  with
    u = O1 - (lam*r1/r2)*O2,  O = E@v unnormalized,  r = rowsum(E);
    subln weight and (1 - lambda_init) folded into Wo on the host;
    rsqrt as exp(-0.5*ln(x)) with the Ln/Exp pairs batched per q-quarter
    after the last attention exp (ACT table loads: ~9 instead of ~33);
    u stored bf16 per q-quarter so o_proj depends on exactly one quarter
  - causal masking: matmuls sliced to valid columns (last diagonal tile
    padded to 256 cols to stay on the fast fp32r path); 0/1 masks applied
    on the gpsimd engine (SBUF-only)
  - kTd duplicate writes ride the gpsimd DGE queue; rope scratch pool opens
    first so its late-draining SBUF range aliases phase B's late-touched
    tiles; p3 runs qi (2,3,1,0) and o_proj's m-groups follow the same
    order so the phase B tail drains under o_proj matmuls
"""

import math
import numpy as np

B, S, D = 2, 2048, 2048
H, KVH = 16, 8
Dh = 64
TP = 4
NCORES = 8
LAYER_IDX = 2
LAMBDA_INIT = 0.8 - 0.6 * math.exp(-0.3 * LAYER_IDX)
EPS = 1e-5
ROPE_THETA = 10000.0

_CACHE = {}


def _build_nc():
    import concourse.bass as bass  # noqa: F401
    import concourse.tile as tile
    from concourse import bacc, mybir

    F32 = mybir.dt.float32
    F32R = mybir.dt.float32r
    BF16 = mybir.dt.bfloat16
    Act = mybir.ActivationFunctionType
    Alu = mybir.AluOpType

    nc = bacc.Bacc("TRN2", target_bir_lowering=False, debug=False)

    xT = nc.dram_tensor("xT", [D, S], BF16, kind="ExternalInput")
    wqT = nc.dram_tensor("wqT", [D, 512], BF16, kind="ExternalInput")
    wkT = nc.dram_tensor("wkT", [D, 256], BF16, kind="ExternalInput")
    wvT = nc.dram_tensor("wvT", [D, 256], BF16, kind="ExternalInput")
    woT = nc.dram_tensor("woT", [512, D], BF16, kind="ExternalInput")
    cosT_d = nc.dram_tensor("cosT", [128, S], F32, kind="ExternalInput")
    ssinT_d = nc.dram_tensor("ssinT", [128, S], F32, kind="ExternalInput")
    tri_d = nc.dram_tensor("tri", [128, 128], BF16, kind="ExternalInput")
    ones_d = nc.dram_tensor("ones", [128, 128], F32R, kind="ExternalInput")
    onesb_d = nc.dram_tensor("onesb", [128, 128], BF16, kind="ExternalInput")
    lam_d = nc.dram_tensor("lam", [128, 1], F32, kind="ExternalInput")
    out_d = nc.dram_tensor("out", [S, D], F32, kind="ExternalOutput")

    KD = D // 128  # 16 contraction tiles

    with tile.TileContext(nc) as tc:
        with tc.tile_pool(name="const", bufs=1) as constp, \
             tc.tile_pool(name="persist", bufs=1) as persist:

            cosT = constp.tile([128, S], F32, tag="cos")
            ssinT = constp.tile([128, S], F32, tag="ssin")
            tri = constp.tile([128, 128], BF16, tag="tri")
            ones = constp.tile([128, 128], F32R, tag="ones")
            onesb = constp.tile([128, 128], BF16, tag="onesb")
            lam = constp.tile([128, 1], F32, tag="lam")

            # per-(head-block, quarter) tiles so phase B's dependencies are
            # exactly the quarter it reads, not the whole projection
            qT_sb = {(m, sh): persist.tile([128, 512], BF16, tag=f"qT{m}_{sh}",
                                           name=f"qT{m}_{sh}")
                     for m in range(4) for sh in range(4)}
            kTd = {(h, sh): persist.tile([128, 512], BF16, tag=f"kTd{h}_{sh}",
                                         name=f"kTd{h}_{sh}")
                   for h in range(4) for sh in range(4)}
            v_sb = [persist.tile([128, 256], BF16, tag=f"v{ms}", name=f"v{ms}")
                    for ms in range(16)]

            # ---------------- Phase A: projections + RoPE ----------------
            # ropet opens first: its SBUF range is the last to drain at the
            # A->B boundary, so it must alias phase B's LATE-touched tiles
            # (uq/pre), not the ET pool that B needs immediately
            with tc.tile_pool(name="ropet", bufs=2) as rp, \
                 tc.tile_pool(name="xtp", bufs=8) as xtp, \
                 tc.tile_pool(name="wres", bufs=1) as wres, \
                 tc.tile_pool(name="psA", bufs=8, space="PSUM") as psA:

                # interleave the sh=0 x tiles with the resident wq tiles so
                # the first q matmul's operands arrive first
                wq_r = [wres.tile([128, 2048], BF16, tag=f"wqr{kp}",
                                  name=f"wqr{kp}") for kp in range(4)]
                xt_t = {}
                for kp in range(4):
                    xt_t[0, kp] = xtp.tile([128, 2048], BF16, tag="xt",
                                           name=f"xt0_{kp}")
                    # first tile pair split in half so the very first matmul's
                    # operands land in ~1.5us instead of ~3us
                    nhalf = 2 if kp == 0 else 1
                    for h in range(nhalf):
                        w = 2048 // nhalf
                        nc.sync.dma_start(
                            out=xt_t[0, kp][:, h * w:(h + 1) * w]
                                .rearrange("p (f n) -> p f n", n=512),
                            in_=xT[kp * 512 + h * (512 // nhalf):
                                   kp * 512 + (h + 1) * (512 // nhalf), 0:512]
                                .rearrange("(f p) n -> p f n", p=128),
                        )
                        nc.sync.dma_start(
                            out=wq_r[kp][:, h * w:(h + 1) * w]
                                .rearrange("p (f n) -> p f n", n=512),
                            in_=wqT[kp * 512 + h * (512 // nhalf):
                                    kp * 512 + (h + 1) * (512 // nhalf), :]
                                .rearrange("(f p) n -> p f n", p=128),
                        )
                wk_r = []
                wv_r = []

                def load_wkv_tables():
                    for kp in range(2):
                        t = wres.tile([128, 2048], BF16, tag=f"wkr{kp}",
                                      name=f"wkr{kp}")
                        nc.sync.dma_start(
                            out=t[:].rearrange("p (eight n) -> p eight n",
                                               eight=8),
                            in_=wkT[kp * 1024:kp * 1024 + 1024, :]
                                .rearrange("(eight p) n -> p eight n", eight=8),
                        )
                        wk_r.append(t)
                        t = wres.tile([128, 2048], BF16, tag=f"wvr{kp}",
                                      name=f"wvr{kp}")
                        nc.sync.dma_start(
                            out=t[:].rearrange("p (eight n) -> p eight n",
                                               eight=8),
                            in_=wvT[kp * 1024:kp * 1024 + 1024, :]
                                .rearrange("(eight p) n -> p eight n", eight=8),
                        )
                        wv_r.append(t)
                    nc.sync.dma_start(out=cosT[:], in_=cosT_d[:])
                    nc.sync.dma_start(out=ssinT[:], in_=ssinT_d[:])
                    nc.sync.dma_start(out=tri[:], in_=tri_d[:])
                    nc.sync.dma_start(out=ones[:], in_=ones_d[:])
                    nc.sync.dma_start(out=onesb[:], in_=onesb_d[:])
                    nc.sync.dma_start(out=lam[:], in_=lam_d[:])

                def wq_lhsT(kd, m):
                    return wq_r[kd // 4][:, (kd % 4) * 512 + m * 128:
                                         (kd % 4) * 512 + m * 128 + 128]

                def wk_lhsT(kd, m):
                    return wk_r[kd // 8][:, (kd % 8) * 256 + m * 128:
                                         (kd % 8) * 256 + m * 128 + 128]

                def wv_rhs(kd):
                    return wv_r[kd // 8][:, (kd % 8) * 256:(kd % 8) * 256 + 256]

                def rope_core(ps, gc0):
                    """RoPE on a [128, 512] psum tile; returns (ra, rt) to add."""
                    gsl = slice(gc0, gc0 + 512)
                    qraw = rp.tile([128, 512], F32, tag="qraw", name="qraw")
                    nc.scalar.copy(qraw[:], ps[:])
                    qsw = rp.tile([128, 512], F32, tag="qsw", name="qsw")
                    for blk in range(4):
                        sb_ = (blk ^ 1) * 32
                        nc.vector.tensor_copy(
                            qsw[blk * 32:blk * 32 + 32, :], qraw[sb_:sb_ + 32, :])
                    nc.vector.tensor_mul(qraw[:], qraw[:], cosT[:, gsl])
                    nc.vector.tensor_mul(qsw[:], qsw[:], ssinT[:, gsl])
                    return qraw, qsw

                def rope_epilogue(ps, dst, gc0):
                    ra, rt = rope_core(ps, gc0)
                    nc.vector.tensor_add(dst[:], ra[:], rt[:])

                def rope_epilogue_kdup(ps, m, sh, gc0):
                    """RoPE then duplicate each 64-row head half into kTd[2m+e]."""
                    ra, rt = rope_core(ps, gc0)
                    ktmp = rp.tile([128, 512], BF16, tag="ktmp", name="ktmp")
                    nc.vector.tensor_add(ktmp[:], ra[:], rt[:])
                    for e in range(2):
                        srcv = ktmp[e * 64:e * 64 + 64, :]
                        nc.gpsimd.dma_start(out=kTd[2 * m + e, sh][0:64, :],
                                            in_=srcv)
                        nc.gpsimd.dma_start(out=kTd[2 * m + e, sh][64:128, :],
                                            in_=srcv)

                for sh in range(4):
                    c0 = 512 * sh
                    if sh > 0:
                        for kp in range(4):
                            xt_t[sh, kp] = xtp.tile([128, 2048], BF16, tag="xt",
                                                    name=f"xt{sh}_{kp}")
                            nc.sync.dma_start(
                                out=xt_t[sh, kp][:]
                                    .rearrange("p (four n) -> p four n", four=4),
                                in_=xT[kp * 512:kp * 512 + 512, c0:c0 + 512]
                                    .rearrange("(four p) n -> p four n", four=4),
                            )

                    def xt_rhs(kd):
                        return xt_t[sh, kd // 4][:, (kd % 4) * 512:
                                                 (kd % 4) * 512 + 512]

                    def emit_qproj(post_mm=None):
                        qps = {}
                        for kd in range(KD):
                            for m in range(4):
                                if kd == 0:
                                    qps[m] = psA.tile([128, 512], F32, tag="pa",
                                                      name=f"qps{m}")
                                nc.tensor.matmul(
                                    qps[m][:],
                                    wq_lhsT(kd, m),
                                    xt_rhs(kd),
                                    start=(kd == 0), stop=(kd == KD - 1),
                                )
                        if post_mm is not None:
                            # table/weight DMAs are emitted here -- after the
                            # q matmuls but BEFORE the ropes that read the
                            # tables (emission order defines dependencies)
                            post_mm()
                        for m in range(4):
                            rope_epilogue(qps[m], qT_sb[m, sh], c0)

                    def emit_kproj():
                        kps = {}
                        for kd in range(KD):
                            for m in range(2):
                                if kd == 0:
                                    kps[m] = psA.tile([128, 512], F32, tag="pa",
                                                      name=f"kps{m}")
                                nc.tensor.matmul(
                                    kps[m][:],
                                    wk_lhsT(kd, m),
                                    xt_rhs(kd),
                                    start=(kd == 0), stop=(kd == KD - 1),
                                )
                        for m in range(2):
                            rope_epilogue_kdup(kps[m], m, sh, c0)


                    def emit_vproj():
                        vps = {}
                        for kd in range(KD):
                            for ms in range(4):
                                if kd == 0:
                                    vps[ms] = psA.tile([128, 512], F32, tag="pa",
                                                       name=f"vps{ms}")
                                nc.tensor.matmul(
                                    vps[ms][:, 0:256],
                                    xt_rhs(kd)[:, ms * 128:ms * 128 + 128],
                                    wv_rhs(kd),
                                    start=(kd == 0), stop=(kd == KD - 1),
                                )
                        for ms in range(4):
                            nc.scalar.copy(v_sb[sh * 4 + ms][:],
                                           vps[ms][:, 0:256])

                    # on the last quarter run k first: its rope+dup chain
                    # feeds phase B and drains under the q/v matmuls
                    if sh == 3:
                        emit_kproj()
                        emit_qproj()
                        emit_vproj()
                    else:
                        emit_qproj(load_wkv_tables if sh == 0 else None)
                        emit_kproj()
                        emit_vproj()

            with tc.tile_pool(name="otfp", bufs=1) as otfp, \
                 tc.tile_pool(name="wop", bufs=1) as wop:
              # u_q[qi][:, p*512:(p+1)*512] holds head-block p's (unscaled,
              # later sf-scaled in place) attention output for q-block qi --
              # one tile per qi so phase C's m-blocks depend only on their
              # own qi group
              u_q = [otfp.tile([128, 2048], BF16, tag=f"uq{qi}",
                               name=f"uq{qi}") for qi in range(4)]
              pre_q = [otfp.tile([128, 2048], BF16, tag=f"preq{qi}",
                                 name=f"preq{qi}") for qi in range(4)]
              sfp = otfp  # sf scratch shares the long-lived pool scope

              def emit_sf_chain(qi, eng="pool"):
                  """rsqrt(pre) and scale u in place: Ln -> Exp(-0.5*) -> mul.
                  The first chain's scale runs split DVE/gpsimd (latency);
                  later chains go to gpsimd so DVE can drain phase C's
                  PSUM->SBUF copies."""
                  sf = sfp.tile([128, 2048], F32, tag="sf", name=f"sf{qi}")
                  nc.scalar.activation(sf[:], pre_q[qi][:], Act.Ln)
                  nc.scalar.activation(sf[:], sf[:], Act.Exp, scale=-0.5)
                  if eng == "split":
                      nc.vector.tensor_mul(u_q[qi][:, 0:1024],
                                           u_q[qi][:, 0:1024], sf[:, 0:1024])
                      nc.gpsimd.tensor_mul(u_q[qi][:, 1024:2048],
                                           u_q[qi][:, 1024:2048],
                                           sf[:, 1024:2048])
                  else:
                      nc.gpsimd.tensor_mul(u_q[qi][:], u_q[qi][:], sf[:])
              # prefetch o_proj weights while phase B runs
              wo_t = {}
              for n in range(4):
                t = wop.tile([128, 2048], BF16, tag=f"wo{n}", name=f"wo{n}")
                nc.sync.dma_start(
                    out=t[:].rearrange("p (four n) -> p four n", four=4),
                    in_=woT[:, n * 512:n * 512 + 512]
                        .rearrange("(four p) n -> p four n", four=4),
                )
                for kc in range(4):
                    wo_t[n, kc] = t[:, kc * 512:kc * 512 + 512]

              # ---------------- Phase B: attention ----------------
              # diag tile j -> first computed column within the q block
              DIAG_C0 = [0, 128, 256, 384]
              # p3 runs qi (2,3,1,0): phase C's m-groups follow the same
              # readiness order, so the last (thin) block's epilogue and the
              # remaining sf chains drain under phase C's first matmuls
              QI_ORDER = {0: (0, 1, 2, 3), 1: (3, 2, 1, 0),
                          2: (3, 2, 1, 0), 3: (2, 3, 1, 0)}
              SHIFT = 3  # accumulate matmuls trail the score matmuls
              with tc.tile_pool(name="etp", bufs=6) as etp, \
                 tc.tile_pool(name="ebp", bufs=7) as ebp, \
                 tc.tile_pool(name="psS", bufs=3, space="PSUM") as psS, \
                 tc.tile_pool(name="psAcc", bufs=4, space="PSUM") as psAcc, \
                 tc.tile_pool(name="psV", bufs=1, space="PSUM") as psV:

                for p in range(4):
                    vh = p // 2
                    for qi in QI_ORDER[p]:
                        q0 = 512 * qi
                        kis = list(range(4 * qi, 4 * qi + 4)) + list(range(4 * qi))
                        nt = len(kis)
                        OT1 = psAcc.tile([128, 512], F32, tag="acc", name="OT1")
                        OT2 = psAcc.tile([128, 512], F32, tag="acc", name="OT2")
                        R1 = psAcc.tile([128, 512], F32, tag="acc", name="R1")
                        R2 = psAcc.tile([128, 512], F32, tag="acc", name="R2")
                        ets = {}

                        def emit_front(t):
                            """scores + exp + mask for k-tile index t."""
                            ki = kis[t]
                            j = ki - 4 * qi
                            vc = DIAG_C0[j] if 0 <= j < 4 else 0
                            S1 = psS.tile([128, 512], F32, tag="s", name="S1")
                            S2 = psS.tile([128, 512], F32, tag="s", name="S2")
                            kt = kTd[p, ki // 4]
                            kcs = slice((ki % 4) * 128, (ki % 4) * 128 + 128)
                            qt = qT_sb[p, qi]
                            nc.tensor.matmul(
                                S1[:, vc:512],
                                kt[0:64, kcs],
                                qt[0:64, vc:512],
                                start=True, stop=True,
                            )
                            nc.tensor.matmul(
                                S2[:, vc:512],
                                kt[64:128, kcs],
                                qt[64:128, vc:512],
                                start=True, stop=True,
                            )
                            ET = etp.tile([128, 1024], BF16, tag="e", name="ET")
                            nc.scalar.activation(ET[:, vc:512], S1[:, vc:512],
                                                 Act.Exp)
                            nc.scalar.activation(ET[:, 512 + vc:1024],
                                                 S2[:, vc:512], Act.Exp)
                            if 0 <= j < 4:
                                msl = slice(vc, vc + 128)
                                nc.gpsimd.tensor_mul(ET[:, msl], ET[:, msl],
                                                     tri[:])
                                nc.gpsimd.tensor_mul(
                                    ET[:, 512 + vc:512 + vc + 128],
                                    ET[:, 512 + vc:512 + vc + 128], tri[:])
                            ets[t] = (ET, vc)

                        def emit_accum(t):
                            """AV + rowsum accumulation for k-tile index t."""
                            ki = kis[t]
                            ET, vc = ets.pop(t)
                            st = (t == 0)
                            sp_ = (t == nt - 1)
                            vt = v_sb[ki][:, vh * 128:vh * 128 + 128]
                            nc.tensor.matmul(OT1[:, vc:512], vt, ET[:, vc:512],
                                             start=st, stop=sp_)
                            nc.tensor.matmul(R1[:, vc:512], onesb[:],
                                             ET[:, vc:512], start=st, stop=sp_)
                            nc.tensor.matmul(OT2[:, vc:512], vt,
                                             ET[:, 512 + vc:1024],
                                             start=st, stop=sp_)
                            nc.tensor.matmul(R2[:, vc:512], onesb[:],
                                             ET[:, 512 + vc:1024],
                                             start=st, stop=sp_)

                        for t in range(nt):
                            emit_front(t)
                            if t >= SHIFT:
                                emit_accum(t - SHIFT)
                        for t in range(max(0, nt - SHIFT), nt):
                            emit_accum(t)

                        if (p, qi) == (3, 0):
                            # last block: emit the first sf chain before this
                            # epilogue so its DVE scale precedes the epilogue
                            # in the DVE queue -- phase C's first m-group
                            # unblocks ~2.5us earlier
                            emit_sf_chain(2, "split")

                        # epilogue: free the PSUM accumulators early (OT1/OT2
                        # copied out on gpsimd), differential combine + RMS
                        # stats; the rsqrt Ln/Exp is deferred past phase B.
                        ot2c = ebp.tile([128, 512], F32, tag="eb", name="ot2c")
                        nc.vector.tensor_copy(ot2c[:], OT2[:])
                        ot1c = ebp.tile([128, 512], F32, tag="eb", name="ot1c")
                        nc.vector.tensor_copy(ot1c[:], OT1[:])
                        rcp2 = ebp.tile([128, 512], F32, tag="eb", name="rcp2")
                        nc.vector.reciprocal(rcp2[:], R2[:])
                        mb = ebp.tile([128, 512], F32, tag="eb", name="mb")
                        nc.vector.scalar_tensor_tensor(
                            mb[:], R1[:], lam[:, 0:1], rcp2[:], Alu.mult, Alu.mult)
                        r1e = ebp.tile([128, 512], F32, tag="eb", name="r1e")
                        nc.vector.tensor_scalar_mul(r1e[:], R1[:],
                                                    math.sqrt(EPS))
                        t2 = ebp.tile([128, 512], F32, tag="eb", name="t2")
                        nc.vector.tensor_mul(t2[:], r1e[:], r1e[:])
                        tt = ebp.tile([128, 512], F32, tag="eb", name="tt")
                        nc.vector.tensor_mul(tt[:], ot2c[:], mb[:])
                        u = u_q[qi][:, p * 512:p * 512 + 512]
                        nc.vector.tensor_sub(u, ot1c[:], tt[:])
                        sq = ebp.tile([128, 512], F32R, tag="eb", name="sq")
                        nc.vector.tensor_mul(sq[:], u, u)
                        varp = psV.tile([128, 512], F32, tag="v", name="varp")
                        nc.tensor.matmul(varp[:], ones[:], sq[:], start=True,
                                         stop=True)
                        nc.vector.scalar_tensor_tensor(
                            pre_q[qi][:, p * 512:p * 512 + 512],
                            varp[:], 1.0 / 128.0, t2[:], Alu.mult, Alu.add)


              # -------- Phase C: o_proj (row-parallel partial) --------
              with tc.tile_pool(name="outp", bufs=6) as outp, \
                 tc.tile_pool(name="psC", bufs=4, space="PSUM") as psC:
                # chain(2) was emitted at the end of phase B; the rest hide
                # under phase C's own matmuls
                for qi in (3, 1, 0):
                    emit_sf_chain(qi, "pool")

                for qi in (2, 3, 1, 0):
                  for mq in range(4):
                    m = qi * 4 + mq
                    for n in range(4):
                        ps = psC.tile([128, 512], F32, tag="pc", name="pc")
                        for kc in range(4):
                            nc.tensor.matmul(
                                ps[:],
                                u_q[qi][:, kc * 512 + mq * 128:
                                        kc * 512 + mq * 128 + 128],
                                wo_t[n, kc],
                                start=(kc == 0), stop=(kc == 3),
                            )
                        # alternate the PSUM->SBUF copy between ACT and DVE
                        # (either alone can't keep pace with the matmuls;
                        # gpsimd cannot read PSUM)
                        cpeng = None
                        last = (m == 3 and n == 3)
                        if not last:
                            ob = outp.tile([128, 512], F32, tag="ob", name="ob")
                            # DVE while the sf chains occupy ACT; later
                            # chunks alternate with ACT
                            if qi in (2, 3) or (m * 4 + n) % 2 == 1:
                                nc.vector.tensor_copy(ob[:], ps[:])
                            else:
                                nc.scalar.copy(ob[:], ps[:])
                            nc.sync.dma_start(
                                out=out_d[m * 128:m * 128 + 128,
                                          n * 512:n * 512 + 512],
                                in_=ob[:])
                        else:
                            # final chunk: halves, so the post-matmul drain is
                            # one short copy+DMA
                            for h in range(2):
                                hs_ = slice(h * 256, h * 256 + 256)
                                ob = outp.tile([128, 512], F32, tag="ob",
                                               name="ob")
                                nc.vector.tensor_copy(ob[:, 0:256],
                                                      ps[:, hs_])
                                nc.sync.dma_start(
                                    out=out_d[m * 128:m * 128 + 128,
                                              n * 512 + h * 256:
                                              n * 512 + h * 256 + 256],
                                    in_=ob[:, 0:256])

    nc.compile()
    return nc


def _host_tables():
    inv = ROPE_THETA ** (-np.arange(Dh, dtype=np.float64) / Dh)
    pos = np.arange(S, dtype=np.float64)
    fr = pos[:, None] * inv[None, :]              # [S, 64]
    cos = np.cos(fr).astype(np.float32)           # [S, 64]
    sin = np.sin(fr).astype(np.float32)
    d = np.arange(128) % 64
    cosT = cos[:, d].T.copy()                     # [128, S]
    sgn = np.where((np.arange(128) % 64) < 32, -1.0, 1.0).astype(np.float32)
    ssinT = (sin[:, d].T * sgn[:, None]).copy()
    tri = np.triu(np.ones((128, 128), np.float32))  # tri[k, q] = 1 if q >= k
    ones = np.ones((128, 128), np.float32)
    return (np.ascontiguousarray(cosT), np.ascontiguousarray(ssinT),
            tri, ones)


def kernel(hidden_states, Wq, Wk, Wv, Wo,
           lambda_q1, lambda_k1, lambda_q2, lambda_k2, subln_weight):
    import ml_dtypes
    from concourse.bass_utils import run_bass_kernel_spmd

    if "nc" not in _CACHE:
        _CACHE["nc"] = _build_nc()
        _CACHE["tables"] = _host_tables()
    nc = _CACHE["nc"]
    cosT, ssinT, tri, ones = _CACHE["tables"]

    f32 = np.float32
    bf16 = ml_dtypes.bfloat16
    hs = np.asarray(hidden_states, f32)
    Wq = np.asarray(Wq, f32)
    Wk = np.asarray(Wk, f32)
    Wv = np.asarray(Wv, f32)
    Wo = np.asarray(Wo, f32)
    subln = np.asarray(subln_weight, f32)

    lam1 = np.exp(np.sum(np.asarray(lambda_q1, f32) * np.asarray(lambda_k1, f32),
                         dtype=f32))
    lam2 = np.exp(np.sum(np.asarray(lambda_q2, f32) * np.asarray(lambda_k2, f32),
                         dtype=f32))
    lam_full = f32(lam1 - lam2 + LAMBDA_INIT)
    lam_arr = np.full((128, 1), lam_full, f32)

    scale = f32(Dh ** -0.5)
    wprime = (np.tile(subln, H) * f32(1.0 - LAMBDA_INIT)).astype(f32)  # [2048]
    WoS = Wo * wprime[None, :]

    in_maps = []
    for c in range(NCORES):
        b, r = c // TP, c % TP
        in_maps.append({
            "xT": np.ascontiguousarray(hs[b].T).astype(bf16),
            "wqT": np.ascontiguousarray(
                (Wq[512 * r:512 * r + 512, :] * scale).T).astype(bf16),
            "wkT": np.ascontiguousarray(
                Wk[256 * r:256 * r + 256, :].T).astype(bf16),
            "wvT": np.ascontiguousarray(
                Wv[256 * r:256 * r + 256, :].T).astype(bf16),
            "woT": np.ascontiguousarray(
                WoS[:, 512 * r:512 * r + 512].T).astype(bf16),
            "cosT": cosT, "ssinT": ssinT, "tri": tri.astype(bf16),
            "ones": ones, "onesb": ones.astype(bf16), "lam": lam_arr,
        })

    res = run_bass_kernel_spmd(nc, in_maps, core_ids=list(range(NCORES)))
    out = np.zeros((B, S, D), f32)
    for c in range(NCORES):
        out[c // TP] += res.results[c]["out"]
    return out
